# revision 14
# baseline (speedup 1.0000x reference)
"""Trainium2 Bass kernel for nn_DglGraphAttentionNetwork (GAT layer over a
random graph, B=16, L=1024, DIN=512, H=4 heads, DH=128).

Strategy (8 NeuronCores, SPMD, two launches + host glue):
  Launch A (data-parallel over nodes): each core projects its 2048 nodes
    h = text @ (W @ fc_w)  (weight product prefolded on host, f32r matmuls)
    and el/er = h . attn_{l,r}. Outputs stay feature-major (tableT [512,2048]
    bf16, elrT [8,2048] f32) so the device does no transposes.
  Host: transposes/concats the 8 table slices, then expands the node table
    into per-edge order (the "gather" is a host permutation): each core
    receives an edge buffer ebuf[block, 128, s_max*512] plus per-edge
    el[src], er[dst] slices. A device dma_gather is descriptor-rate-bound
    (~8ns/row on GpSimd), while plain DMA streams at the full 360GB/s.
  Launch B (dst-sharded): 128-dst blocks, 16 per core. Per block: DMA the
    edge rows, build one-hot dst masks with 4x-mode tensor_scalar(is_equal),
    compute per-edge softmax weights w = exp(leaky(el+er)) on ACT, weight
    the messages on DVE (rh = w*h), and accumulate per-destination sums and
    denominators as masked matmuls in PSUM.
"""

import os
import sys

sys.path.insert(0, "/opt/trn_rl_repo")

from contextlib import ExitStack

import numpy as np
import ml_dtypes

import jax
from jax.sharding import Mesh, PartitionSpec
from jax.experimental.shard_map import shard_map

try:
    jax.config.update("jax_compilation_cache_dir", "/tmp/gat_jax_cache")
    jax.config.update("jax_persistent_cache_min_compile_time_secs", 1.0)
    jax.config.update("jax_persistent_cache_min_entry_size_bytes", -1)
except Exception:
    pass

import concourse.bass as bass
import concourse.bacc as bacc
import concourse.mybir as mybir
import concourse.tile as tile
from concourse.bass2jax import _bass_exec_p, install_neuronx_cc_hook, partition_id_tensor

F32 = mybir.dt.float32
F32R = mybir.dt.float32r
BF16 = mybir.dt.bfloat16
BF16NP = ml_dtypes.bfloat16

B, L, DIN = 16, 1024, 512
H, DH = 4, 128
N = B * L           # 16384 nodes
NC = 8              # cores
NPC = N // NC       # 2048 nodes per core
NBLK = 128          # destination blocks of 128 nodes
BPC = NBLK // NC    # 16 blocks per core
NEG = 0.2           # leaky_relu slope
FEAT = H * DH       # 512

ACT = mybir.ActivationFunctionType
ALU = mybir.AluOpType


# ----------------------------------------------------------------------------
# Launch A: projection. Per core: textT [512, 2048] -> tableT [512, 2048] bf16,
# elrT [8, 2048] f32.
# ----------------------------------------------------------------------------

def build_phase_a():
    nc = bacc.Bacc("TRN2", target_bir_lowering=False, debug=False,
                   enable_asserts=False, num_devices=NC)
    textT = nc.dram_tensor("textT", [DIN, NPC], BF16, kind="ExternalInput").ap()
    wfc = nc.dram_tensor("wfc", [DIN, FEAT], BF16, kind="ExternalInput").ap()
    attnb = nc.dram_tensor("attnb", [DIN, 2 * H], BF16, kind="ExternalInput").ap()
    biasT = nc.dram_tensor("biasT", [128, 4], F32, kind="ExternalInput").ap()
    elrc = nc.dram_tensor("elrc", [2 * H, 1], F32, kind="ExternalInput").ap()
    tableT = nc.dram_tensor("tableT", [FEAT, NPC], BF16, kind="ExternalOutput").ap()
    elrT = nc.dram_tensor("elrT", [2 * H, NPC], F32, kind="ExternalOutput").ap()

    KT = DIN // 128    # 4 contraction tiles
    NCH = NPC // 512   # 4 node chunks of 512

    with tile.TileContext(nc) as tc, ExitStack() as ctx:
        wpool = ctx.enter_context(tc.tile_pool(name="w", bufs=1))
        cpool = ctx.enter_context(tc.tile_pool(name="c", bufs=2))
        hpool = ctx.enter_context(tc.tile_pool(name="h", bufs=2))
        pmm = ctx.enter_context(tc.tile_pool(name="pmm", bufs=4, space="PSUM"))
        pelr = ctx.enter_context(tc.tile_pool(name="pelr", bufs=2, space="PSUM"))

        # bf16 matmuls tolerate mixed producers: DMA loads feed PE directly
        w_sb = [wpool.tile([128, FEAT], BF16, tag=f"w{i}", name=f"w{i}")
                for i in range(KT)]
        for i in range(KT):
            nc.gpsimd.dma_start(w_sb[i][:], wfc[i * 128:(i + 1) * 128, :])
        attn_sb = wpool.tile([128, KT, 2 * H], BF16, tag="at", name="at")
        nc.gpsimd.dma_start(attn_sb[:],
                            attnb.rearrange("(f p) h -> p f h", p=128))
        biasT_sb = wpool.tile([128, 4], F32, tag="bt", name="bt")
        nc.gpsimd.dma_start(biasT_sb[:], biasT[:])
        elrc_sb = wpool.tile([2 * H, 1], F32, tag="ec", name="ec")
        nc.gpsimd.dma_start(elrc_sb[:], elrc[:])

        for nchk in range(NCH):
            c0 = nchk * 512
            tT_sb = [cpool.tile([128, 512], BF16, tag=f"tt{i}", name=f"tt{i}")
                     for i in range(KT)]
            for i in range(KT):
                nc.gpsimd.dma_start(
                    tT_sb[i][:], textT[i * 128:(i + 1) * 128, c0:c0 + 512])

            # hT[f, n] = sum_d wfc[d, f] * textT[d, n] ; emit bf16 per ft tile
            hb = [cpool.tile([128, 512], BF16, tag=f"hb{i}", name=f"hb{i}")
                  for i in range(KT)]
            for ft in range(KT):
                p = pmm.tile([128, 512], F32, tag="pmm", name="pmm")
                for dt in range(KT):
                    nc.tensor.matmul(
                        p[:],
                        w_sb[dt][:, ft * 128:(ft + 1) * 128],
                        tT_sb[dt][:],
                        start=(dt == 0), stop=(dt == KT - 1))
                nc.scalar.activation(hb[ft][:], p[:], ACT.Identity,
                                     bias=biasT_sb[:, ft:ft + 1])
                nc.gpsimd.dma_start(
                    tableT[ft * 128:(ft + 1) * 128, c0:c0 + 512], hb[ft][:])

            # elrT[c, n] = sum_f attn[f, c] * hT[f, n]
            pe = pelr.tile([2 * H, 512], F32, tag="pelr", name="pelr")
            for ft in range(KT):
                nc.tensor.matmul(
                    pe[:], attn_sb[:, ft, :], hb[ft][:],
                    start=(ft == 0), stop=(ft == KT - 1))
            elr_sb = hpool.tile([2 * H, 512], F32, tag="elr", name="elr")
            nc.vector.tensor_scalar(elr_sb[:], pe[:], elrc_sb[:], None,
                                    op0=ALU.subtract)
            nc.gpsimd.dma_start(elrT[:, c0:c0 + 512], elr_sb[:])
    nc.compile()
    return nc


# ----------------------------------------------------------------------------
# Launch B: edge-softmax aggregation, dst-sharded.
# ----------------------------------------------------------------------------

def build_phase_b(s_max: int):
    SM = s_max

    nc = bacc.Bacc("TRN2", target_bir_lowering=False, debug=False,
                   enable_asserts=False, num_devices=NC)
    ebuf = nc.dram_tensor("ebuf", [BPC * 128, SM * FEAT], BF16,
                          kind="ExternalInput").ap()
    dcol_c = nc.dram_tensor("dcolc", [128, BPC * SM], BF16,
                            kind="ExternalInput").ap()
    iota_r = nc.dram_tensor("iotar", [128, 128], BF16,
                            kind="ExternalInput").ap()
    el_in = nc.dram_tensor("elin", [128, BPC * SM * H], BF16,
                           kind="ExternalInput").ap()
    er_in = nc.dram_tensor("erin", [128, BPC * SM * H], BF16,
                           kind="ExternalInput").ap()
    out = nc.dram_tensor("out", [NPC, FEAT], BF16, kind="ExternalOutput").ap()
    I32 = mybir.dt.int32

    with tile.TileContext(nc) as tc, ExitStack() as ctx:
        cpool = ctx.enter_context(tc.tile_pool(name="c", bufs=1))
        gpool = ctx.enter_context(tc.tile_pool(name="g", bufs=3))
        mpool = ctx.enter_context(tc.tile_pool(name="m", bufs=3))
        rpool = ctx.enter_context(tc.tile_pool(name="r", bufs=2))
        wpool = ctx.enter_context(tc.tile_pool(name="wk", bufs=3))
        opool = ctx.enter_context(tc.tile_pool(name="o", bufs=2))
        pfeat = ctx.enter_context(tc.tile_pool(name="pf", bufs=3, space="PSUM"))
        pden = ctx.enter_context(tc.tile_pool(name="pd", bufs=3, space="PSUM"))

        dc_sb = cpool.tile([128, BPC * SM], BF16, tag="dc", name="dc")
        nc.sync.dma_start(dc_sb[:], dcol_c[:])
        ior_sb = cpool.tile([128, 128], BF16, tag="ior", name="ior")
        nc.sync.dma_start(ior_sb[:], iota_r[:])
        el_sb = cpool.tile([128, BPC, SM, H], BF16, tag="el", name="el")
        nc.sync.dma_start(el_sb[:], el_in.rearrange("p (b s h) -> p b s h",
                                                    b=BPC, s=SM))
        er_sb = cpool.tile([128, BPC, SM, H], BF16, tag="er", name="er")
        nc.sync.dma_start(er_sb[:], er_in.rearrange("p (b s h) -> p b s h",
                                                    b=BPC, s=SM))

        # per-edge weights w = exp(leaky_relu(el[src] + er[dst])), all blocks
        # at once, written twice (packed pairs) so wx can broadcast as int32
        e_all = cpool.tile([128, BPC, SM, H], BF16, tag="e", name="e")
        nc.vector.tensor_tensor(e_all[:], el_sb[:], er_sb[:], op=ALU.add)
        lk_all = cpool.tile([128, BPC, SM, H], BF16, tag="lk", name="lk")
        nc.vector.tensor_scalar_mul(lk_all[:], e_all[:], NEG)
        nc.vector.tensor_max(lk_all[:], lk_all[:], e_all[:])
        wg2 = cpool.tile([128, BPC, SM, H, 2], BF16, tag="wg", name="wg")
        for rep in range(2):
            nc.scalar.activation(wg2[:, :, :, :, rep], lk_all[:], ACT.Exp)

        def block_front(b):
            g_sb = gpool.tile([128, SM, FEAT], BF16, tag="g", name="g")
            nc.gpsimd.dma_start(
                g_sb[:], ebuf[b * 128:(b + 1) * 128, :].rearrange(
                    "p (s f) -> p s f", s=SM))
            # one-hot dst masks (single 1x-mode is_equal per block)
            m_sb = mpool.tile([128, SM, 128], BF16, tag="m", name="m")
            nc.vector.tensor_tensor(
                m_sb[:],
                dc_sb[:, b * SM:(b + 1) * SM].unsqueeze(2)
                    .to_broadcast((128, SM, 128)),
                ior_sb[:].unsqueeze(1).to_broadcast((128, SM, 128)),
                op=ALU.is_equal)

            # materialize w densely on the otherwise-idle GpSimd engine:
            # broadcast-copy the packed bf16 pairs as int32
            wx = rpool.tile([128, SM, H, DH], BF16, tag="wx", name="wx")
            wgi = wg2[:, b].bitcast(I32)
            nc.gpsimd.tensor_copy(
                wx[:].bitcast(I32),
                wgi.to_broadcast((128, SM, H, DH // 2)))
            rh = rpool.tile([128, SM, FEAT], BF16, tag="rh", name="rh")
            nc.vector.tensor_tensor(
                rh[:], g_sb[:], wx[:].rearrange("a s h d -> a s (h d)"),
                op=ALU.mult)

            # masked-matmul aggregation + denominators
            pf = pfeat.tile([128, FEAT], F32, tag="pf", name="pf")
            pd = pden.tile([128, H], F32, tag="pd", name="pd")
            for sbt in range(SM):
                st, sp = (sbt == 0), (sbt == SM - 1)
                nc.tensor.matmul(pf[:], m_sb[:, sbt, :], rh[:, sbt],
                                 start=st, stop=sp)
                nc.tensor.matmul(pd[:], m_sb[:, sbt, :], wg2[:, b, sbt, :, 0],
                                 start=st, stop=sp)
            return pf, pd

        def block_epilogue(b, pf, pd):
            den_sb = wpool.tile([128, H], F32, tag="den", name="den")
            nc.scalar.activation(den_sb[:], pd[:], ACT.Copy)
            rec_sb = wpool.tile([128, H], F32, tag="rec", name="rec")
            nc.vector.reciprocal(rec_sb[:], den_sb[:])
            o_sb = opool.tile([128, FEAT], BF16, tag="o", name="o")
            for h in range(H):
                nc.scalar.activation(
                    o_sb[:, h * DH:(h + 1) * DH], pf[:, h * DH:(h + 1) * DH],
                    ACT.Copy, scale=rec_sb[:, h:h + 1])
            nc.gpsimd.dma_start(out[b * 128:(b + 1) * 128, :], o_sb[:])

        # software pipeline: block b's epilogue is emitted after block b+1's
        # front so no engine stream stalls on the PSUM accumulation
        prev = None
        for b in range(BPC):
            cur = block_front(b)
            if prev is not None:
                block_epilogue(b - 1, *prev)
            prev = cur
        block_epilogue(BPC - 1, *prev)
    nc.compile()
    return nc


# ----------------------------------------------------------------------------
# Host side
# ----------------------------------------------------------------------------

def _preprocess(src, dst):
    """Relabel nodes so 128-dst blocks are edge-balanced; build per-edge
    block layouts (edge position = subtile*128 + partition)."""
    deg = np.bincount(dst, minlength=N)
    order = np.argsort(-deg, kind="stable")
    ranks = np.arange(N)
    rounds, pos = ranks // NBLK, ranks % NBLK
    blk = np.where(rounds % 2 == 0, pos, NBLK - 1 - pos)
    new_id = np.empty(N, np.int64)
    new_id[order] = blk * 128 + rounds
    bsum = np.bincount(new_id[dst] // 128, minlength=NBLK)
    s_max = int(np.ceil(bsum.max() / 128))
    p_b = s_max * 128
    s2, d2 = new_id[src], new_id[dst]
    eo = np.argsort(d2, kind="stable")
    s2, d2 = s2[eo], d2[eo]
    starts = np.concatenate([[0], np.cumsum(bsum)])
    eblk = d2 // 128
    flatpos = eblk * p_b + (np.arange(len(d2)) - starts[eblk])
    bsrc = np.zeros(NBLK * p_b, np.int64)
    bsrc[flatpos] = s2
    bdst = np.zeros(NBLK * p_b, np.int64)
    bdst[flatpos] = d2
    bcol = np.full(NBLK * p_b, 255.0, np.float32)
    bcol[flatpos] = (d2 % 128).astype(np.float32)
    return (new_id, bsrc.reshape(NBLK, p_b), bdst.reshape(NBLK, p_b),
            bcol.reshape(NBLK, p_b), s_max)


_CACHE = {}


class _Runner:
    """Cached SPMD runner: jits the bass_exec body once per Bass module."""

    def __init__(self, nc):
        install_neuronx_cc_hook()
        self.nc = nc
        part_name = (nc.partition_id_tensor.name
                     if nc.partition_id_tensor else None)
        in_names, out_names, out_avals, zero_outs = [], [], [], []
        for alloc in nc.m.functions[0].allocations:
            if not isinstance(alloc, mybir.MemoryLocationSet):
                continue
            name = alloc.memorylocations[0].name
            if alloc.kind == "ExternalInput":
                if name != part_name:
                    in_names.append(name)
            elif alloc.kind == "ExternalOutput":
                out_names.append(name)
                shape = tuple(alloc.tensor_shape)
                dtype = mybir.dt.np(alloc.dtype)
                out_avals.append(jax.core.ShapedArray(shape, dtype))
                zero_outs.append(np.zeros(shape, dtype))
        self.in_names, self.out_names = in_names, out_names
        self.out_avals, self.zero_outs = out_avals, zero_outs
        n_params, n_outs = len(in_names), len(out_avals)
        all_names = tuple(in_names + out_names
                          + ([part_name] if part_name else []))
        avals = tuple(out_avals)

        def _body(*args):
            operands = list(args)
            if part_name is not None:
                operands.append(partition_id_tensor())
            outs = _bass_exec_p.bind(
                *operands,
                out_avals=avals,
                in_names=all_names,
                out_names=tuple(out_names),
                lowering_input_output_aliases=(),
                sim_require_finite=True,
                sim_require_nnan=True,
                nc=nc,
            )
            return tuple(outs)

        devices = jax.devices()[:NC]
        self.mesh = Mesh(np.asarray(devices), ("core",))
        in_specs = (PartitionSpec("core"),) * (n_params + n_outs)
        out_specs = (PartitionSpec("core"),) * n_outs
        self.fn = jax.jit(
            shard_map(_body, mesh=self.mesh, in_specs=in_specs,
                      out_specs=out_specs, check_rep=False),
            keep_unused=True)

    def prep(self, in_maps):
        """Concatenate per-core inputs along axis 0 (host)."""
        n_params = len(self.in_names)
        concat_in = [
            np.concatenate([in_maps[c][self.in_names[i]] for c in range(NC)],
                           axis=0)
            for i in range(n_params)]
        concat_zeros = [
            np.zeros((NC * z.shape[0], *z.shape[1:]), z.dtype)
            for z in self.zero_outs]
        return concat_in + concat_zeros

    def run_prepped(self, args):
        return self.fn(*args)

    def run(self, in_maps):
        out_arrs = self.fn(*self.prep(in_maps))
        return [
            {name: np.asarray(out_arrs[i]).reshape(NC, *self.out_avals[i].shape)[c]
             for i, name in enumerate(self.out_names)}
            for c in range(NC)]


def _get_kernels(s_max):
    if s_max not in _CACHE:
        _CACHE[s_max] = (_Runner(build_phase_a()), _Runner(build_phase_b(s_max)))
    return _CACHE[s_max]


def kernel(text, weight, fc_w, attn_l, attn_r, bias, src, dst):
    text = np.asarray(text, np.float32)
    weight = np.asarray(weight, np.float32)
    fc_w = np.asarray(fc_w, np.float32)
    attn_l = np.asarray(attn_l, np.float32)
    attn_r = np.asarray(attn_r, np.float32)
    bias = np.asarray(bias, np.float32)
    src = np.asarray(src).astype(np.int64)
    dst = np.asarray(dst).astype(np.int64)

    new_id, bsrc, bdst, bcol, s_max = _preprocess(src, dst)
    orig_for_new = np.empty(N, np.int64)
    orig_for_new[new_id] = np.arange(N)

    run_a, run_b = _get_kernels(s_max)

    # --- launch A ---
    wfc = (weight.astype(np.float64) @ fc_w.astype(np.float64)).astype(BF16NP)
    attn_cat = np.zeros((DIN, 2 * H), np.float32)
    for h in range(H):
        attn_cat[h * DH:(h + 1) * DH, h] = attn_l[h]
        attn_cat[h * DH:(h + 1) * DH, H + h] = attn_r[h]
    attn_b = attn_cat.astype(BF16NP)
    biasT_h = np.ascontiguousarray(bias.reshape(4, 128).T, dtype=np.float32)
    elrc_h = (bias @ attn_cat).reshape(2 * H, 1).astype(np.float32)
    text_flat = text.reshape(N, DIN)
    in_maps_a = []
    for c in range(NC):
        rows = orig_for_new[c * NPC:(c + 1) * NPC]
        textT = np.ascontiguousarray(text_flat[rows].T).astype(BF16NP)
        in_maps_a.append({"textT": textT, "wfc": wfc, "attnb": attn_b,
                          "biasT": biasT_h, "elrc": elrc_h})
    res_a = run_a.run(in_maps_a)

    # node-major table / el / er in new-id space
    table_full = np.concatenate(
        [np.ascontiguousarray(r["tableT"].T) for r in res_a], axis=0)
    elr_full = np.concatenate([r["elrT"].T for r in res_a], axis=0)
    el_full = np.ascontiguousarray(elr_full[:, :H]).astype(BF16NP)
    er_full = np.ascontiguousarray(elr_full[:, H:]).astype(BF16NP)

    # --- host expansion: node table -> per-edge buffers ---
    # bsrc[blk, s*128+p] -> layout [blk, p, s]
    idx_ps = bsrc.reshape(NBLK, s_max, 128).transpose(0, 2, 1)
    ebuf_all = table_full[idx_ps].reshape(NBLK, 128, s_max * FEAT)
    el_e = el_full[idx_ps]                               # [NBLK,128,s_max,H]
    er_e = er_full[bdst.reshape(NBLK, s_max, 128).transpose(0, 2, 1)]
    iota_row = np.broadcast_to(
        np.arange(128, dtype=np.float32), (128, 128)).astype(BF16NP)

    in_maps_b = []
    for c in range(NC):
        blks = slice(c * BPC, (c + 1) * BPC)
        dcolc = np.concatenate(
            [bcol[b].reshape(s_max, 128).T
             for b in range(c * BPC, (c + 1) * BPC)], axis=1)
        dcolc = np.ascontiguousarray(dcolc).astype(BF16NP)
        elin = np.ascontiguousarray(
            el_e[blks].transpose(1, 0, 2, 3).reshape(128, BPC * s_max * H))
        erin = np.ascontiguousarray(
            er_e[blks].transpose(1, 0, 2, 3).reshape(128, BPC * s_max * H))
        in_maps_b.append({
            "ebuf": ebuf_all[blks].reshape(BPC * 128, s_max * FEAT),
            "dcolc": dcolc, "iotar": iota_row,
            "elin": elin, "erin": erin})
    res_b = run_b.run(in_maps_b)

    out_new = np.concatenate([r["out"].astype(np.float32) for r in res_b],
                             axis=0)
    result = out_new[new_id].reshape(B, L, H * DH).astype(np.float32)

    global _LAST_ARGS
    _LAST_ARGS = (run_a, in_maps_a, run_b, in_maps_b)
    return result


_LAST_ARGS = None


# revision 16
# speedup vs baseline: 1.7063x; 1.7063x over previous
"""Trainium2 Bass kernel for nn_DglGraphAttentionNetwork (GAT layer over a
random graph, B=16, L=1024, DIN=512, H=4 heads, DH=128).

Strategy (8 NeuronCores, SPMD, two launches + host glue):
  Launch A (data-parallel over nodes): each core projects its 2048 nodes
    h = text @ (W @ fc_w)  (weight product prefolded on host, f32r matmuls)
    and el/er = h . attn_{l,r}. Outputs stay feature-major (tableT [512,2048]
    bf16, elrT [8,2048] f32) so the device does no transposes.
  Host: transposes/concats the 8 table slices, then expands the node table
    into per-edge order (the "gather" is a host permutation): each core
    receives an edge buffer ebuf[block, 128, s_max*512] plus per-edge
    el[src], er[dst] slices. A device dma_gather is descriptor-rate-bound
    (~8ns/row on GpSimd), while plain DMA streams at the full 360GB/s.
  Launch B (dst-sharded): 128-dst blocks, 16 per core. Per block: DMA the
    edge rows, build one-hot dst masks with 4x-mode tensor_scalar(is_equal),
    compute per-edge softmax weights w = exp(leaky(el+er)) on ACT, weight
    the messages on DVE (rh = w*h), and accumulate per-destination sums and
    denominators as masked matmuls in PSUM.
"""

import os
import sys

sys.path.insert(0, "/opt/trn_rl_repo")

from contextlib import ExitStack

import numpy as np
import ml_dtypes

import jax
from jax.sharding import Mesh, PartitionSpec
from jax.experimental.shard_map import shard_map

try:
    jax.config.update("jax_compilation_cache_dir", "/tmp/gat_jax_cache")
    jax.config.update("jax_persistent_cache_min_compile_time_secs", 1.0)
    jax.config.update("jax_persistent_cache_min_entry_size_bytes", -1)
except Exception:
    pass

import concourse.bass as bass
import concourse.bacc as bacc
import concourse.mybir as mybir
import concourse.tile as tile
from concourse.bass2jax import _bass_exec_p, install_neuronx_cc_hook, partition_id_tensor

F32 = mybir.dt.float32
F32R = mybir.dt.float32r
BF16 = mybir.dt.bfloat16
BF16NP = ml_dtypes.bfloat16

B, L, DIN = 16, 1024, 512
H, DH = 4, 128
N = B * L           # 16384 nodes
NC = 8              # cores
NPC = N // NC       # 2048 nodes per core
NBLK = 128          # destination blocks of 128 nodes
BPC = NBLK // NC    # 16 blocks per core
NEG = 0.2           # leaky_relu slope
FEAT = H * DH       # 512

ACT = mybir.ActivationFunctionType
ALU = mybir.AluOpType


# ----------------------------------------------------------------------------
# Launch A: projection. Per core: textT [512, 2048] -> tableT [512, 2048] bf16,
# elrT [8, 2048] f32.
# ----------------------------------------------------------------------------

def build_phase_a():
    nc = bacc.Bacc("TRN2", target_bir_lowering=False, debug=False,
                   enable_asserts=False, num_devices=NC)
    textT = nc.dram_tensor("textT", [DIN, NPC], BF16, kind="ExternalInput").ap()
    wfc = nc.dram_tensor("wfc", [DIN, FEAT], BF16, kind="ExternalInput").ap()
    attnb = nc.dram_tensor("attnb", [DIN, 2 * H], BF16, kind="ExternalInput").ap()
    biasT = nc.dram_tensor("biasT", [128, 4], F32, kind="ExternalInput").ap()
    elrc = nc.dram_tensor("elrc", [2 * H, 1], F32, kind="ExternalInput").ap()
    tableT = nc.dram_tensor("tableT", [FEAT, NPC], BF16, kind="ExternalOutput").ap()
    elrT = nc.dram_tensor("elrT", [2 * H, NPC], F32, kind="ExternalOutput").ap()

    KT = DIN // 128    # 4 contraction tiles
    NCH = NPC // 512   # 4 node chunks of 512

    with tile.TileContext(nc) as tc, ExitStack() as ctx:
        wpool = ctx.enter_context(tc.tile_pool(name="w", bufs=1))
        cpool = ctx.enter_context(tc.tile_pool(name="c", bufs=2))
        hpool = ctx.enter_context(tc.tile_pool(name="h", bufs=2))
        pmm = ctx.enter_context(tc.tile_pool(name="pmm", bufs=4, space="PSUM"))
        pelr = ctx.enter_context(tc.tile_pool(name="pelr", bufs=2, space="PSUM"))

        # bf16 matmuls tolerate mixed producers: DMA loads feed PE directly
        w_sb = [wpool.tile([128, FEAT], BF16, tag=f"w{i}", name=f"w{i}")
                for i in range(KT)]
        for i in range(KT):
            nc.gpsimd.dma_start(w_sb[i][:], wfc[i * 128:(i + 1) * 128, :])
        attn_sb = wpool.tile([128, KT, 2 * H], BF16, tag="at", name="at")
        nc.gpsimd.dma_start(attn_sb[:],
                            attnb.rearrange("(f p) h -> p f h", p=128))
        biasT_sb = wpool.tile([128, 4], F32, tag="bt", name="bt")
        nc.gpsimd.dma_start(biasT_sb[:], biasT[:])
        elrc_sb = wpool.tile([2 * H, 1], F32, tag="ec", name="ec")
        nc.gpsimd.dma_start(elrc_sb[:], elrc[:])

        for nchk in range(NCH):
            c0 = nchk * 512
            tT_sb = [cpool.tile([128, 512], BF16, tag=f"tt{i}", name=f"tt{i}")
                     for i in range(KT)]
            for i in range(KT):
                nc.gpsimd.dma_start(
                    tT_sb[i][:], textT[i * 128:(i + 1) * 128, c0:c0 + 512])

            # hT[f, n] = sum_d wfc[d, f] * textT[d, n] ; emit bf16 per ft tile
            hb = [cpool.tile([128, 512], BF16, tag=f"hb{i}", name=f"hb{i}")
                  for i in range(KT)]
            for ft in range(KT):
                p = pmm.tile([128, 512], F32, tag="pmm", name="pmm")
                for dt in range(KT):
                    nc.tensor.matmul(
                        p[:],
                        w_sb[dt][:, ft * 128:(ft + 1) * 128],
                        tT_sb[dt][:],
                        start=(dt == 0), stop=(dt == KT - 1))
                nc.scalar.activation(hb[ft][:], p[:], ACT.Identity,
                                     bias=biasT_sb[:, ft:ft + 1])
                nc.gpsimd.dma_start(
                    tableT[ft * 128:(ft + 1) * 128, c0:c0 + 512], hb[ft][:])

            # elrT[c, n] = sum_f attn[f, c] * hT[f, n]
            pe = pelr.tile([2 * H, 512], F32, tag="pelr", name="pelr")
            for ft in range(KT):
                nc.tensor.matmul(
                    pe[:], attn_sb[:, ft, :], hb[ft][:],
                    start=(ft == 0), stop=(ft == KT - 1))
            elr_sb = hpool.tile([2 * H, 512], F32, tag="elr", name="elr")
            nc.vector.tensor_scalar(elr_sb[:], pe[:], elrc_sb[:], None,
                                    op0=ALU.subtract)
            nc.gpsimd.dma_start(elrT[:, c0:c0 + 512], elr_sb[:])
    nc.compile()
    return nc


# ----------------------------------------------------------------------------
# Launch B: edge-softmax aggregation, dst-sharded.
# ----------------------------------------------------------------------------

def build_phase_b(s_max: int):
    SM = s_max

    nc = bacc.Bacc("TRN2", target_bir_lowering=False, debug=False,
                   enable_asserts=False, num_devices=NC)
    ebuf = nc.dram_tensor("ebuf", [BPC * 128, SM * FEAT], BF16,
                          kind="ExternalInput").ap()
    msk_in = nc.dram_tensor("msk", [BPC * 128, SM * 128], BF16,
                            kind="ExternalInput").ap()
    el_in = nc.dram_tensor("elin", [128, BPC * SM * H], BF16,
                           kind="ExternalInput").ap()
    er_in = nc.dram_tensor("erin", [128, BPC * SM * H], BF16,
                           kind="ExternalInput").ap()
    out = nc.dram_tensor("out", [NPC, FEAT], BF16, kind="ExternalOutput").ap()
    I32 = mybir.dt.int32

    with tile.TileContext(nc) as tc, ExitStack() as ctx:
        cpool = ctx.enter_context(tc.tile_pool(name="c", bufs=1))
        gpool = ctx.enter_context(tc.tile_pool(name="g", bufs=4))
        mpool = ctx.enter_context(tc.tile_pool(name="m", bufs=3))
        rpool = ctx.enter_context(tc.tile_pool(name="r", bufs=2))
        wpool = ctx.enter_context(tc.tile_pool(name="wk", bufs=3))
        opool = ctx.enter_context(tc.tile_pool(name="o", bufs=2))
        pfeat = ctx.enter_context(tc.tile_pool(name="pf", bufs=3, space="PSUM"))
        pden = ctx.enter_context(tc.tile_pool(name="pd", bufs=3, space="PSUM"))

        el_sb = cpool.tile([128, BPC, SM, H], BF16, tag="el", name="el")
        nc.sync.dma_start(el_sb[:], el_in.rearrange("p (b s h) -> p b s h",
                                                    b=BPC, s=SM))
        er_sb = cpool.tile([128, BPC, SM, H], BF16, tag="er", name="er")
        nc.sync.dma_start(er_sb[:], er_in.rearrange("p (b s h) -> p b s h",
                                                    b=BPC, s=SM))

        # per-edge weights w = exp(leaky_relu(el[src] + er[dst])), all blocks
        # at once, written twice (packed pairs) so wx can broadcast as int32
        e_all = cpool.tile([128, BPC, SM, H], BF16, tag="e", name="e")
        nc.vector.tensor_tensor(e_all[:], el_sb[:], er_sb[:], op=ALU.add)
        lk_all = cpool.tile([128, BPC, SM, H], BF16, tag="lk", name="lk")
        nc.vector.tensor_scalar_mul(lk_all[:], e_all[:], NEG)
        nc.vector.tensor_max(lk_all[:], lk_all[:], e_all[:])
        wg2 = cpool.tile([128, BPC, SM, H, 2], BF16, tag="wg", name="wg")
        for rep in range(2):
            nc.scalar.activation(wg2[:, :, :, :, rep], lk_all[:], ACT.Exp)

        def block_front(b):
            g_sb = gpool.tile([128, SM, FEAT], BF16, tag="g", name="g")
            nc.gpsimd.dma_start(
                g_sb[:], ebuf[b * 128:(b + 1) * 128, :].rearrange(
                    "p (s f) -> p s f", s=SM))
            # one-hot dst masks, precomputed on host
            m_sb = mpool.tile([128, SM, 128], BF16, tag="m", name="m")
            nc.sync.dma_start(
                m_sb[:], msk_in[b * 128:(b + 1) * 128, :].rearrange(
                    "p (s j) -> p s j", s=SM))

            # materialize w densely on the scalar engine (packed-int32
            # broadcast copy), freeing the DVE for the big multiply
            wx = rpool.tile([128, SM, H, DH], BF16, tag="wx", name="wx")
            nc.scalar.activation(
                wx[:], wg2[:, b, :, :, 0:1].to_broadcast((128, SM, H, DH)),
                ACT.Copy)
            rh = rpool.tile([128, SM, FEAT], BF16, tag="rh", name="rh")
            nc.vector.tensor_tensor(
                rh[:], g_sb[:], wx[:].rearrange("a s h d -> a s (h d)"),
                op=ALU.mult)

            # masked-matmul aggregation + denominators
            pf = pfeat.tile([128, FEAT], F32, tag="pf", name="pf")
            pd = pden.tile([128, H], F32, tag="pd", name="pd")
            for sbt in range(SM):
                st, sp = (sbt == 0), (sbt == SM - 1)
                nc.tensor.matmul(pf[:], m_sb[:, sbt, :], rh[:, sbt],
                                 start=st, stop=sp)
                nc.tensor.matmul(pd[:], m_sb[:, sbt, :], wg2[:, b, sbt, :, 0],
                                 start=st, stop=sp)
            return pf, pd

        def block_epilogue(b, pf, pd):
            den_sb = wpool.tile([128, H], F32, tag="den", name="den")
            nc.scalar.activation(den_sb[:], pd[:], ACT.Copy)
            rec_sb = wpool.tile([128, H], F32, tag="rec", name="rec")
            nc.vector.reciprocal(rec_sb[:], den_sb[:])
            o_sb = opool.tile([128, FEAT], BF16, tag="o", name="o")
            for h in range(H):
                nc.scalar.activation(
                    o_sb[:, h * DH:(h + 1) * DH], pf[:, h * DH:(h + 1) * DH],
                    ACT.Copy, scale=rec_sb[:, h:h + 1])
            nc.gpsimd.dma_start(out[b * 128:(b + 1) * 128, :], o_sb[:])

        # software pipeline: block b's epilogue is emitted after block b+1's
        # front so no engine stream stalls on the PSUM accumulation
        prev = None
        for b in range(BPC):
            cur = block_front(b)
            if prev is not None:
                block_epilogue(b - 1, *prev)
            prev = cur
        block_epilogue(BPC - 1, *prev)
    nc.compile()
    return nc


# ----------------------------------------------------------------------------
# Host side
# ----------------------------------------------------------------------------

def _preprocess(src, dst):
    """Relabel nodes so 128-dst blocks are edge-balanced; build per-edge
    block layouts (edge position = subtile*128 + partition)."""
    deg = np.bincount(dst, minlength=N)
    order = np.argsort(-deg, kind="stable")
    ranks = np.arange(N)
    rounds, pos = ranks // NBLK, ranks % NBLK
    blk = np.where(rounds % 2 == 0, pos, NBLK - 1 - pos)
    new_id = np.empty(N, np.int64)
    new_id[order] = blk * 128 + rounds
    bsum = np.bincount(new_id[dst] // 128, minlength=NBLK)
    s_max = int(np.ceil(bsum.max() / 128))
    p_b = s_max * 128
    s2, d2 = new_id[src], new_id[dst]
    eo = np.argsort(d2, kind="stable")
    s2, d2 = s2[eo], d2[eo]
    starts = np.concatenate([[0], np.cumsum(bsum)])
    eblk = d2 // 128
    flatpos = eblk * p_b + (np.arange(len(d2)) - starts[eblk])
    bsrc = np.zeros(NBLK * p_b, np.int64)
    bsrc[flatpos] = s2
    bdst = np.zeros(NBLK * p_b, np.int64)
    bdst[flatpos] = d2
    bcol = np.full(NBLK * p_b, 255.0, np.float32)
    bcol[flatpos] = (d2 % 128).astype(np.float32)
    return (new_id, bsrc.reshape(NBLK, p_b), bdst.reshape(NBLK, p_b),
            bcol.reshape(NBLK, p_b), s_max)


_CACHE = {}


class _Runner:
    """Cached SPMD runner: jits the bass_exec body once per Bass module."""

    def __init__(self, nc):
        install_neuronx_cc_hook()
        self.nc = nc
        part_name = (nc.partition_id_tensor.name
                     if nc.partition_id_tensor else None)
        in_names, out_names, out_avals, zero_outs = [], [], [], []
        for alloc in nc.m.functions[0].allocations:
            if not isinstance(alloc, mybir.MemoryLocationSet):
                continue
            name = alloc.memorylocations[0].name
            if alloc.kind == "ExternalInput":
                if name != part_name:
                    in_names.append(name)
            elif alloc.kind == "ExternalOutput":
                out_names.append(name)
                shape = tuple(alloc.tensor_shape)
                dtype = mybir.dt.np(alloc.dtype)
                out_avals.append(jax.core.ShapedArray(shape, dtype))
                zero_outs.append(np.zeros(shape, dtype))
        self.in_names, self.out_names = in_names, out_names
        self.out_avals, self.zero_outs = out_avals, zero_outs
        n_params, n_outs = len(in_names), len(out_avals)
        all_names = tuple(in_names + out_names
                          + ([part_name] if part_name else []))
        avals = tuple(out_avals)

        def _body(*args):
            operands = list(args)
            if part_name is not None:
                operands.append(partition_id_tensor())
            outs = _bass_exec_p.bind(
                *operands,
                out_avals=avals,
                in_names=all_names,
                out_names=tuple(out_names),
                lowering_input_output_aliases=(),
                sim_require_finite=True,
                sim_require_nnan=True,
                nc=nc,
            )
            return tuple(outs)

        devices = jax.devices()[:NC]
        self.mesh = Mesh(np.asarray(devices), ("core",))
        in_specs = (PartitionSpec("core"),) * (n_params + n_outs)
        out_specs = (PartitionSpec("core"),) * n_outs
        self.fn = jax.jit(
            shard_map(_body, mesh=self.mesh, in_specs=in_specs,
                      out_specs=out_specs, check_rep=False),
            keep_unused=True)

    def prep(self, in_maps):
        """Concatenate per-core inputs along axis 0 (host)."""
        n_params = len(self.in_names)
        concat_in = [
            np.concatenate([in_maps[c][self.in_names[i]] for c in range(NC)],
                           axis=0)
            for i in range(n_params)]
        concat_zeros = [
            np.zeros((NC * z.shape[0], *z.shape[1:]), z.dtype)
            for z in self.zero_outs]
        return concat_in + concat_zeros

    def run_prepped(self, args):
        return self.fn(*args)

    def run(self, in_maps):
        out_arrs = self.fn(*self.prep(in_maps))
        return [
            {name: np.asarray(out_arrs[i]).reshape(NC, *self.out_avals[i].shape)[c]
             for i, name in enumerate(self.out_names)}
            for c in range(NC)]


def _get_kernels(s_max):
    if s_max not in _CACHE:
        _CACHE[s_max] = (_Runner(build_phase_a()), _Runner(build_phase_b(s_max)))
    return _CACHE[s_max]


def kernel(text, weight, fc_w, attn_l, attn_r, bias, src, dst):
    text = np.asarray(text, np.float32)
    weight = np.asarray(weight, np.float32)
    fc_w = np.asarray(fc_w, np.float32)
    attn_l = np.asarray(attn_l, np.float32)
    attn_r = np.asarray(attn_r, np.float32)
    bias = np.asarray(bias, np.float32)
    src = np.asarray(src).astype(np.int64)
    dst = np.asarray(dst).astype(np.int64)

    new_id, bsrc, bdst, bcol, s_max = _preprocess(src, dst)
    orig_for_new = np.empty(N, np.int64)
    orig_for_new[new_id] = np.arange(N)

    run_a, run_b = _get_kernels(s_max)

    # --- launch A ---
    wfc = (weight.astype(np.float64) @ fc_w.astype(np.float64)).astype(BF16NP)
    attn_cat = np.zeros((DIN, 2 * H), np.float32)
    for h in range(H):
        attn_cat[h * DH:(h + 1) * DH, h] = attn_l[h]
        attn_cat[h * DH:(h + 1) * DH, H + h] = attn_r[h]
    attn_b = attn_cat.astype(BF16NP)
    biasT_h = np.ascontiguousarray(bias.reshape(4, 128).T, dtype=np.float32)
    elrc_h = (bias @ attn_cat).reshape(2 * H, 1).astype(np.float32)
    text_flat = text.reshape(N, DIN)
    in_maps_a = []
    for c in range(NC):
        rows = orig_for_new[c * NPC:(c + 1) * NPC]
        textT = np.ascontiguousarray(text_flat[rows].T).astype(BF16NP)
        in_maps_a.append({"textT": textT, "wfc": wfc, "attnb": attn_b,
                          "biasT": biasT_h, "elrc": elrc_h})
    res_a = run_a.run(in_maps_a)

    # node-major table / el / er in new-id space
    table_full = np.concatenate(
        [np.ascontiguousarray(r["tableT"].T) for r in res_a], axis=0)
    elr_full = np.concatenate([r["elrT"].T for r in res_a], axis=0)
    el_full = np.ascontiguousarray(elr_full[:, :H]).astype(BF16NP)
    er_full = np.ascontiguousarray(elr_full[:, H:]).astype(BF16NP)

    # --- host expansion: node table -> per-edge buffers ---
    # bsrc[blk, s*128+p] -> layout [blk, p, s]
    idx_ps = bsrc.reshape(NBLK, s_max, 128).transpose(0, 2, 1)
    ebuf_all = table_full[idx_ps].reshape(NBLK, 128, s_max * FEAT)
    el_e = el_full[idx_ps]                               # [NBLK,128,s_max,H]
    er_e = er_full[bdst.reshape(NBLK, s_max, 128).transpose(0, 2, 1)]
    # one-hot dst masks [blk, p, s, j]
    msk_all = (bcol.reshape(NBLK, s_max, 128).transpose(0, 2, 1)[:, :, :, None]
               == np.arange(128, dtype=np.float32)).astype(BF16NP)
    msk_all = msk_all.reshape(NBLK, 128, s_max * 128)

    in_maps_b = []
    for c in range(NC):
        blks = slice(c * BPC, (c + 1) * BPC)
        elin = np.ascontiguousarray(
            el_e[blks].transpose(1, 0, 2, 3).reshape(128, BPC * s_max * H))
        erin = np.ascontiguousarray(
            er_e[blks].transpose(1, 0, 2, 3).reshape(128, BPC * s_max * H))
        in_maps_b.append({
            "ebuf": ebuf_all[blks].reshape(BPC * 128, s_max * FEAT),
            "msk": msk_all[blks].reshape(BPC * 128, s_max * 128),
            "elin": elin, "erin": erin})
    res_b = run_b.run(in_maps_b)

    out_new = np.concatenate([r["out"].astype(np.float32) for r in res_b],
                             axis=0)
    result = out_new[new_id].reshape(B, L, H * DH).astype(np.float32)

    global _LAST_ARGS
    _LAST_ARGS = (run_a, in_maps_a, run_b, in_maps_b)
    return result


_LAST_ARGS = None


# revision 18
# speedup vs baseline: 1.9571x; 1.1470x over previous
"""Trainium2 Bass kernel for nn_DglGraphAttentionNetwork (GAT layer over a
random graph, B=16, L=1024, DIN=512, H=4 heads, DH=128).

Strategy (8 NeuronCores, SPMD, two launches + host glue):
  Launch A (data-parallel over nodes): each core projects its 2048 nodes
    h = text @ (W @ fc_w)  (weight product prefolded on host, f32r matmuls)
    and el/er = h . attn_{l,r}. Outputs stay feature-major (tableT [512,2048]
    bf16, elrT [8,2048] f32) so the device does no transposes.
  Host: transposes/concats the 8 table slices, then expands the node table
    into per-edge order (the "gather" is a host permutation): each core
    receives an edge buffer ebuf[block, 128, s_max*512] plus per-edge
    el[src], er[dst] slices. A device dma_gather is descriptor-rate-bound
    (~8ns/row on GpSimd), while plain DMA streams at the full 360GB/s.
  Launch B (dst-sharded): 128-dst blocks, 16 per core. Per block: DMA the
    edge rows, build one-hot dst masks with 4x-mode tensor_scalar(is_equal),
    compute per-edge softmax weights w = exp(leaky(el+er)) on ACT, weight
    the messages on DVE (rh = w*h), and accumulate per-destination sums and
    denominators as masked matmuls in PSUM.
"""

import os
import sys

sys.path.insert(0, "/opt/trn_rl_repo")

from contextlib import ExitStack

import numpy as np
import ml_dtypes

import jax
from jax.sharding import Mesh, PartitionSpec
from jax.experimental.shard_map import shard_map

try:
    jax.config.update("jax_compilation_cache_dir", "/tmp/gat_jax_cache")
    jax.config.update("jax_persistent_cache_min_compile_time_secs", 1.0)
    jax.config.update("jax_persistent_cache_min_entry_size_bytes", -1)
except Exception:
    pass

import concourse.bass as bass
import concourse.bacc as bacc
import concourse.mybir as mybir
import concourse.tile as tile
from concourse.bass2jax import _bass_exec_p, install_neuronx_cc_hook, partition_id_tensor

F32 = mybir.dt.float32
F32R = mybir.dt.float32r
BF16 = mybir.dt.bfloat16
BF16NP = ml_dtypes.bfloat16

B, L, DIN = 16, 1024, 512
H, DH = 4, 128
N = B * L           # 16384 nodes
NC = 8              # cores
NPC = N // NC       # 2048 nodes per core
NBLK = 128          # destination blocks of 128 nodes
BPC = NBLK // NC    # 16 blocks per core
NEG = 0.2           # leaky_relu slope
FEAT = H * DH       # 512

ACT = mybir.ActivationFunctionType
ALU = mybir.AluOpType


# ----------------------------------------------------------------------------
# Launch A: projection. Per core: textT [512, 2048] -> tableT [512, 2048] bf16,
# elrT [8, 2048] f32.
# ----------------------------------------------------------------------------

def build_phase_a():
    nc = bacc.Bacc("TRN2", target_bir_lowering=False, debug=False,
                   enable_asserts=False, num_devices=NC)
    textT = nc.dram_tensor("textT", [DIN, NPC], BF16, kind="ExternalInput").ap()
    wfc = nc.dram_tensor("wfc", [DIN, FEAT], BF16, kind="ExternalInput").ap()
    attnb = nc.dram_tensor("attnb", [DIN, 2 * H], BF16, kind="ExternalInput").ap()
    biasT = nc.dram_tensor("biasT", [128, 4], F32, kind="ExternalInput").ap()
    elrc = nc.dram_tensor("elrc", [2 * H, 1], F32, kind="ExternalInput").ap()
    tableT = nc.dram_tensor("tableT", [FEAT, NPC], BF16, kind="ExternalOutput").ap()
    elrT = nc.dram_tensor("elrT", [2 * H, NPC], F32, kind="ExternalOutput").ap()

    KT = DIN // 128    # 4 contraction tiles
    NCH = NPC // 512   # 4 node chunks of 512

    with tile.TileContext(nc) as tc, ExitStack() as ctx:
        wpool = ctx.enter_context(tc.tile_pool(name="w", bufs=1))
        cpool = ctx.enter_context(tc.tile_pool(name="c", bufs=2))
        hpool = ctx.enter_context(tc.tile_pool(name="h", bufs=2))
        pmm = ctx.enter_context(tc.tile_pool(name="pmm", bufs=4, space="PSUM"))
        pelr = ctx.enter_context(tc.tile_pool(name="pelr", bufs=2, space="PSUM"))

        # bf16 matmuls tolerate mixed producers: DMA loads feed PE directly
        w_sb = [wpool.tile([128, FEAT], BF16, tag=f"w{i}", name=f"w{i}")
                for i in range(KT)]
        for i in range(KT):
            nc.gpsimd.dma_start(w_sb[i][:], wfc[i * 128:(i + 1) * 128, :])
        attn_sb = wpool.tile([128, KT, 2 * H], BF16, tag="at", name="at")
        nc.gpsimd.dma_start(attn_sb[:],
                            attnb.rearrange("(f p) h -> p f h", p=128))
        biasT_sb = wpool.tile([128, 4], F32, tag="bt", name="bt")
        nc.gpsimd.dma_start(biasT_sb[:], biasT[:])
        elrc_sb = wpool.tile([2 * H, 1], F32, tag="ec", name="ec")
        nc.gpsimd.dma_start(elrc_sb[:], elrc[:])

        for nchk in range(NCH):
            c0 = nchk * 512
            tT_sb = [cpool.tile([128, 512], BF16, tag=f"tt{i}", name=f"tt{i}")
                     for i in range(KT)]
            for i in range(KT):
                nc.gpsimd.dma_start(
                    tT_sb[i][:], textT[i * 128:(i + 1) * 128, c0:c0 + 512])

            # hT[f, n] = sum_d wfc[d, f] * textT[d, n] ; emit bf16 per ft tile
            hb = [cpool.tile([128, 512], BF16, tag=f"hb{i}", name=f"hb{i}")
                  for i in range(KT)]
            for ft in range(KT):
                p = pmm.tile([128, 512], F32, tag="pmm", name="pmm")
                for dt in range(KT):
                    nc.tensor.matmul(
                        p[:],
                        w_sb[dt][:, ft * 128:(ft + 1) * 128],
                        tT_sb[dt][:],
                        start=(dt == 0), stop=(dt == KT - 1))
                nc.scalar.activation(hb[ft][:], p[:], ACT.Identity,
                                     bias=biasT_sb[:, ft:ft + 1])
                nc.gpsimd.dma_start(
                    tableT[ft * 128:(ft + 1) * 128, c0:c0 + 512], hb[ft][:])

            # elrT[c, n] = sum_f attn[f, c] * hT[f, n]
            pe = pelr.tile([2 * H, 512], F32, tag="pelr", name="pelr")
            for ft in range(KT):
                nc.tensor.matmul(
                    pe[:], attn_sb[:, ft, :], hb[ft][:],
                    start=(ft == 0), stop=(ft == KT - 1))
            elr_sb = hpool.tile([2 * H, 512], F32, tag="elr", name="elr")
            nc.vector.tensor_scalar(elr_sb[:], pe[:], elrc_sb[:], None,
                                    op0=ALU.subtract)
            nc.gpsimd.dma_start(elrT[:, c0:c0 + 512], elr_sb[:])
    nc.compile()
    return nc


# ----------------------------------------------------------------------------
# Launch B: edge-softmax aggregation, dst-sharded.
# ----------------------------------------------------------------------------

def build_phase_b(s_max: int):
    SM = s_max

    nc = bacc.Bacc("TRN2", target_bir_lowering=False, debug=False,
                   enable_asserts=False, num_devices=NC)
    ebuf = nc.dram_tensor("ebuf", [BPC * 128, SM * FEAT], BF16,
                          kind="ExternalInput").ap()
    FP8 = mybir.dt.float8e4
    msk_in = nc.dram_tensor("msk", [BPC * 128, SM * 128], FP8,
                            kind="ExternalInput").ap()
    el_in = nc.dram_tensor("elin", [128, BPC * SM * H], BF16,
                           kind="ExternalInput").ap()
    er_in = nc.dram_tensor("erin", [128, BPC * SM * H], BF16,
                           kind="ExternalInput").ap()
    out = nc.dram_tensor("out", [NPC, FEAT], BF16, kind="ExternalOutput").ap()
    I32 = mybir.dt.int32

    with tile.TileContext(nc) as tc, ExitStack() as ctx:
        cpool = ctx.enter_context(tc.tile_pool(name="c", bufs=1))
        gpool = ctx.enter_context(tc.tile_pool(name="g", bufs=4))
        mpool = ctx.enter_context(tc.tile_pool(name="m", bufs=3))
        rpool = ctx.enter_context(tc.tile_pool(name="r", bufs=2))
        wpool = ctx.enter_context(tc.tile_pool(name="wk", bufs=3))
        opool = ctx.enter_context(tc.tile_pool(name="o", bufs=2))
        pfeat = ctx.enter_context(tc.tile_pool(name="pf", bufs=3, space="PSUM"))
        pden = ctx.enter_context(tc.tile_pool(name="pd", bufs=3, space="PSUM"))

        el_sb = cpool.tile([128, BPC, SM, H], BF16, tag="el", name="el")
        nc.sync.dma_start(el_sb[:], el_in.rearrange("p (b s h) -> p b s h",
                                                    b=BPC, s=SM))
        er_sb = cpool.tile([128, BPC, SM, H], BF16, tag="er", name="er")
        nc.sync.dma_start(er_sb[:], er_in.rearrange("p (b s h) -> p b s h",
                                                    b=BPC, s=SM))

        # per-edge weights w = exp(leaky_relu(el[src] + er[dst])), all blocks
        # at once, written twice (packed pairs) so wx can broadcast as int32
        e_all = cpool.tile([128, BPC, SM, H], BF16, tag="e", name="e")
        nc.vector.tensor_tensor(e_all[:], el_sb[:], er_sb[:], op=ALU.add)
        lk_all = cpool.tile([128, BPC, SM, H], BF16, tag="lk", name="lk")
        nc.vector.tensor_scalar_mul(lk_all[:], e_all[:], NEG)
        nc.vector.tensor_max(lk_all[:], lk_all[:], e_all[:])
        wg2 = cpool.tile([128, BPC, SM, H, 2], BF16, tag="wg", name="wg")
        for rep in range(2):
            nc.scalar.activation(wg2[:, :, :, :, rep], lk_all[:], ACT.Exp)

        def block_front(b):
            g_sb = gpool.tile([128, SM, FEAT], BF16, tag="g", name="g")
            nc.gpsimd.dma_start(
                g_sb[:], ebuf[b * 128:(b + 1) * 128, :].rearrange(
                    "p (s f) -> p s f", s=SM))
            # one-hot dst masks, precomputed on host
            m_sb = mpool.tile([128, SM, 128], FP8, tag="m", name="m")
            nc.sync.dma_start(
                m_sb[:], msk_in[b * 128:(b + 1) * 128, :].rearrange(
                    "p (s j) -> p s j", s=SM))

            # materialize w densely on the scalar engine (packed-int32
            # broadcast copy), freeing the DVE for the big multiply
            wx = rpool.tile([128, SM, H, DH], BF16, tag="wx", name="wx")
            wgi = wg2[:, b].bitcast(I32)
            nc.vector.tensor_copy(
                wx[:].bitcast(I32),
                wgi.to_broadcast((128, SM, H, DH // 2)))
            rh = rpool.tile([128, SM, FEAT], BF16, tag="rh", name="rh")
            nc.vector.tensor_tensor(
                rh[:], g_sb[:], wx[:].rearrange("a s h d -> a s (h d)"),
                op=ALU.mult)

            # masked-matmul aggregation + denominators
            pf = pfeat.tile([128, FEAT], F32, tag="pf", name="pf")
            pd = pden.tile([128, H], F32, tag="pd", name="pd")
            for sbt in range(SM):
                st, sp = (sbt == 0), (sbt == SM - 1)
                nc.tensor.matmul(pf[:], m_sb[:, sbt, :], rh[:, sbt],
                                 start=st, stop=sp)
                nc.tensor.matmul(pd[:], m_sb[:, sbt, :], wg2[:, b, sbt, :, 0],
                                 start=st, stop=sp)
            return pf, pd

        def block_epilogue(b, pf, pd):
            den_sb = wpool.tile([128, H], F32, tag="den", name="den")
            nc.scalar.activation(den_sb[:], pd[:], ACT.Copy)
            rec_sb = wpool.tile([128, H], F32, tag="rec", name="rec")
            nc.vector.reciprocal(rec_sb[:], den_sb[:])
            o_sb = opool.tile([128, FEAT], BF16, tag="o", name="o")
            for h in range(H):
                nc.scalar.activation(
                    o_sb[:, h * DH:(h + 1) * DH], pf[:, h * DH:(h + 1) * DH],
                    ACT.Copy, scale=rec_sb[:, h:h + 1])
            nc.gpsimd.dma_start(out[b * 128:(b + 1) * 128, :], o_sb[:])

        # software pipeline: block b's epilogue is emitted after block b+1's
        # front so no engine stream stalls on the PSUM accumulation
        prev = None
        for b in range(BPC):
            cur = block_front(b)
            if prev is not None:
                block_epilogue(b - 1, *prev)
            prev = cur
        block_epilogue(BPC - 1, *prev)
    nc.compile()
    return nc


# ----------------------------------------------------------------------------
# Host side
# ----------------------------------------------------------------------------

def _preprocess(src, dst):
    """Relabel nodes so 128-dst blocks are edge-balanced; build per-edge
    block layouts (edge position = subtile*128 + partition)."""
    deg = np.bincount(dst, minlength=N)
    order = np.argsort(-deg, kind="stable")
    ranks = np.arange(N)
    rounds, pos = ranks // NBLK, ranks % NBLK
    blk = np.where(rounds % 2 == 0, pos, NBLK - 1 - pos)
    new_id = np.empty(N, np.int64)
    new_id[order] = blk * 128 + rounds
    bsum = np.bincount(new_id[dst] // 128, minlength=NBLK)
    s_max = int(np.ceil(bsum.max() / 128))
    p_b = s_max * 128
    s2, d2 = new_id[src], new_id[dst]
    eo = np.argsort(d2, kind="stable")
    s2, d2 = s2[eo], d2[eo]
    starts = np.concatenate([[0], np.cumsum(bsum)])
    eblk = d2 // 128
    flatpos = eblk * p_b + (np.arange(len(d2)) - starts[eblk])
    bsrc = np.zeros(NBLK * p_b, np.int64)
    bsrc[flatpos] = s2
    bdst = np.zeros(NBLK * p_b, np.int64)
    bdst[flatpos] = d2
    bcol = np.full(NBLK * p_b, 255.0, np.float32)
    bcol[flatpos] = (d2 % 128).astype(np.float32)
    return (new_id, bsrc.reshape(NBLK, p_b), bdst.reshape(NBLK, p_b),
            bcol.reshape(NBLK, p_b), s_max)


_CACHE = {}


class _Runner:
    """Cached SPMD runner: jits the bass_exec body once per Bass module."""

    def __init__(self, nc):
        install_neuronx_cc_hook()
        self.nc = nc
        part_name = (nc.partition_id_tensor.name
                     if nc.partition_id_tensor else None)
        in_names, out_names, out_avals, zero_outs = [], [], [], []
        for alloc in nc.m.functions[0].allocations:
            if not isinstance(alloc, mybir.MemoryLocationSet):
                continue
            name = alloc.memorylocations[0].name
            if alloc.kind == "ExternalInput":
                if name != part_name:
                    in_names.append(name)
            elif alloc.kind == "ExternalOutput":
                out_names.append(name)
                shape = tuple(alloc.tensor_shape)
                dtype = mybir.dt.np(alloc.dtype)
                out_avals.append(jax.core.ShapedArray(shape, dtype))
                zero_outs.append(np.zeros(shape, dtype))
        self.in_names, self.out_names = in_names, out_names
        self.out_avals, self.zero_outs = out_avals, zero_outs
        n_params, n_outs = len(in_names), len(out_avals)
        all_names = tuple(in_names + out_names
                          + ([part_name] if part_name else []))
        avals = tuple(out_avals)

        def _body(*args):
            operands = list(args)
            if part_name is not None:
                operands.append(partition_id_tensor())
            outs = _bass_exec_p.bind(
                *operands,
                out_avals=avals,
                in_names=all_names,
                out_names=tuple(out_names),
                lowering_input_output_aliases=(),
                sim_require_finite=True,
                sim_require_nnan=True,
                nc=nc,
            )
            return tuple(outs)

        devices = jax.devices()[:NC]
        self.mesh = Mesh(np.asarray(devices), ("core",))
        in_specs = (PartitionSpec("core"),) * (n_params + n_outs)
        out_specs = (PartitionSpec("core"),) * n_outs
        self.fn = jax.jit(
            shard_map(_body, mesh=self.mesh, in_specs=in_specs,
                      out_specs=out_specs, check_rep=False),
            keep_unused=True)

    def prep(self, in_maps):
        """Concatenate per-core inputs along axis 0 (host)."""
        n_params = len(self.in_names)
        concat_in = [
            np.concatenate([in_maps[c][self.in_names[i]] for c in range(NC)],
                           axis=0)
            for i in range(n_params)]
        concat_zeros = [
            np.zeros((NC * z.shape[0], *z.shape[1:]), z.dtype)
            for z in self.zero_outs]
        return concat_in + concat_zeros

    def run_prepped(self, args):
        return self.fn(*args)

    def run(self, in_maps):
        out_arrs = self.fn(*self.prep(in_maps))
        return [
            {name: np.asarray(out_arrs[i]).reshape(NC, *self.out_avals[i].shape)[c]
             for i, name in enumerate(self.out_names)}
            for c in range(NC)]


def _get_kernels(s_max):
    if s_max not in _CACHE:
        _CACHE[s_max] = (_Runner(build_phase_a()), _Runner(build_phase_b(s_max)))
    return _CACHE[s_max]


def kernel(text, weight, fc_w, attn_l, attn_r, bias, src, dst):
    text = np.asarray(text, np.float32)
    weight = np.asarray(weight, np.float32)
    fc_w = np.asarray(fc_w, np.float32)
    attn_l = np.asarray(attn_l, np.float32)
    attn_r = np.asarray(attn_r, np.float32)
    bias = np.asarray(bias, np.float32)
    src = np.asarray(src).astype(np.int64)
    dst = np.asarray(dst).astype(np.int64)

    new_id, bsrc, bdst, bcol, s_max = _preprocess(src, dst)
    orig_for_new = np.empty(N, np.int64)
    orig_for_new[new_id] = np.arange(N)

    run_a, run_b = _get_kernels(s_max)

    # --- launch A ---
    wfc = (weight.astype(np.float64) @ fc_w.astype(np.float64)).astype(BF16NP)
    attn_cat = np.zeros((DIN, 2 * H), np.float32)
    for h in range(H):
        attn_cat[h * DH:(h + 1) * DH, h] = attn_l[h]
        attn_cat[h * DH:(h + 1) * DH, H + h] = attn_r[h]
    attn_b = attn_cat.astype(BF16NP)
    biasT_h = np.ascontiguousarray(bias.reshape(4, 128).T, dtype=np.float32)
    elrc_h = (bias @ attn_cat).reshape(2 * H, 1).astype(np.float32)
    text_flat = text.reshape(N, DIN)
    in_maps_a = []
    for c in range(NC):
        rows = orig_for_new[c * NPC:(c + 1) * NPC]
        textT = np.ascontiguousarray(text_flat[rows].T).astype(BF16NP)
        in_maps_a.append({"textT": textT, "wfc": wfc, "attnb": attn_b,
                          "biasT": biasT_h, "elrc": elrc_h})
    res_a = run_a.run(in_maps_a)

    # node-major table / el / er in new-id space
    table_full = np.concatenate(
        [np.ascontiguousarray(r["tableT"].T) for r in res_a], axis=0)
    elr_full = np.concatenate([r["elrT"].T for r in res_a], axis=0)
    el_full = np.ascontiguousarray(elr_full[:, :H]).astype(BF16NP)
    er_full = np.ascontiguousarray(elr_full[:, H:]).astype(BF16NP)

    # --- host expansion: node table -> per-edge buffers ---
    # bsrc[blk, s*128+p] -> layout [blk, p, s]
    idx_ps = bsrc.reshape(NBLK, s_max, 128).transpose(0, 2, 1)
    ebuf_all = table_full[idx_ps].reshape(NBLK, 128, s_max * FEAT)
    el_e = el_full[idx_ps]                               # [NBLK,128,s_max,H]
    er_e = er_full[bdst.reshape(NBLK, s_max, 128).transpose(0, 2, 1)]
    # one-hot dst masks [blk, p, s, j]
    msk_all = (bcol.reshape(NBLK, s_max, 128).transpose(0, 2, 1)[:, :, :, None]
               == np.arange(128, dtype=np.float32)).astype(ml_dtypes.float8_e4m3)
    msk_all = msk_all.reshape(NBLK, 128, s_max * 128)

    in_maps_b = []
    for c in range(NC):
        blks = slice(c * BPC, (c + 1) * BPC)
        elin = np.ascontiguousarray(
            el_e[blks].transpose(1, 0, 2, 3).reshape(128, BPC * s_max * H))
        erin = np.ascontiguousarray(
            er_e[blks].transpose(1, 0, 2, 3).reshape(128, BPC * s_max * H))
        in_maps_b.append({
            "ebuf": ebuf_all[blks].reshape(BPC * 128, s_max * FEAT),
            "msk": msk_all[blks].reshape(BPC * 128, s_max * 128),
            "elin": elin, "erin": erin})
    res_b = run_b.run(in_maps_b)

    out_new = np.concatenate([r["out"].astype(np.float32) for r in res_b],
                             axis=0)
    result = out_new[new_id].reshape(B, L, H * DH).astype(np.float32)

    global _LAST_ARGS
    _LAST_ARGS = (run_a, in_maps_a, run_b, in_maps_b)
    return result


_LAST_ARGS = None


# revision 19
# speedup vs baseline: 1.9882x; 1.0159x over previous
"""Trainium2 Bass kernel for nn_DglGraphAttentionNetwork (GAT layer over a
random graph, B=16, L=1024, DIN=512, H=4 heads, DH=128).

Strategy (8 NeuronCores, SPMD, two launches + host glue):
  Launch A (data-parallel over nodes): each core projects its 2048 nodes
    h = text @ (W @ fc_w)  (weight product prefolded on host, f32r matmuls)
    and el/er = h . attn_{l,r}. Outputs stay feature-major (tableT [512,2048]
    bf16, elrT [8,2048] f32) so the device does no transposes.
  Host: transposes/concats the 8 table slices, then expands the node table
    into per-edge order (the "gather" is a host permutation): each core
    receives an edge buffer ebuf[block, 128, s_max*512] plus per-edge
    el[src], er[dst] slices. A device dma_gather is descriptor-rate-bound
    (~8ns/row on GpSimd), while plain DMA streams at the full 360GB/s.
  Launch B (dst-sharded): 128-dst blocks, 16 per core. Per block: DMA the
    edge rows, build one-hot dst masks with 4x-mode tensor_scalar(is_equal),
    compute per-edge softmax weights w = exp(leaky(el+er)) on ACT, weight
    the messages on DVE (rh = w*h), and accumulate per-destination sums and
    denominators as masked matmuls in PSUM.
"""

import os
import sys

sys.path.insert(0, "/opt/trn_rl_repo")

from contextlib import ExitStack

import numpy as np
import ml_dtypes

import jax
from jax.sharding import Mesh, PartitionSpec
from jax.experimental.shard_map import shard_map

try:
    jax.config.update("jax_compilation_cache_dir", "/tmp/gat_jax_cache")
    jax.config.update("jax_persistent_cache_min_compile_time_secs", 1.0)
    jax.config.update("jax_persistent_cache_min_entry_size_bytes", -1)
except Exception:
    pass

import concourse.bass as bass
import concourse.bacc as bacc
import concourse.mybir as mybir
import concourse.tile as tile
from concourse.bass2jax import _bass_exec_p, install_neuronx_cc_hook, partition_id_tensor

F32 = mybir.dt.float32
F32R = mybir.dt.float32r
BF16 = mybir.dt.bfloat16
BF16NP = ml_dtypes.bfloat16

B, L, DIN = 16, 1024, 512
H, DH = 4, 128
N = B * L           # 16384 nodes
NC = 8              # cores
NPC = N // NC       # 2048 nodes per core
NBLK = 128          # destination blocks of 128 nodes
BPC = NBLK // NC    # 16 blocks per core
NEG = 0.2           # leaky_relu slope
FEAT = H * DH       # 512

ACT = mybir.ActivationFunctionType
ALU = mybir.AluOpType


# ----------------------------------------------------------------------------
# Launch A: projection. Per core: textT [512, 2048] -> tableT [512, 2048] bf16,
# elrT [8, 2048] f32.
# ----------------------------------------------------------------------------

def build_phase_a():
    nc = bacc.Bacc("TRN2", target_bir_lowering=False, debug=False,
                   enable_asserts=False, num_devices=NC)
    textT = nc.dram_tensor("textT", [DIN, NPC], BF16, kind="ExternalInput").ap()
    wfc = nc.dram_tensor("wfc", [DIN, FEAT], BF16, kind="ExternalInput").ap()
    attnb = nc.dram_tensor("attnb", [DIN, 2 * H], BF16, kind="ExternalInput").ap()
    biasT = nc.dram_tensor("biasT", [128, 4], F32, kind="ExternalInput").ap()
    elrc = nc.dram_tensor("elrc", [2 * H, 1], F32, kind="ExternalInput").ap()
    tableT = nc.dram_tensor("tableT", [FEAT, NPC], BF16, kind="ExternalOutput").ap()
    elrT = nc.dram_tensor("elrT", [2 * H, NPC], F32, kind="ExternalOutput").ap()

    KT = DIN // 128    # 4 contraction tiles
    NCH = NPC // 512   # 4 node chunks of 512

    with tile.TileContext(nc) as tc, ExitStack() as ctx:
        wpool = ctx.enter_context(tc.tile_pool(name="w", bufs=1))
        cpool = ctx.enter_context(tc.tile_pool(name="c", bufs=2))
        hpool = ctx.enter_context(tc.tile_pool(name="h", bufs=2))
        pmm = ctx.enter_context(tc.tile_pool(name="pmm", bufs=4, space="PSUM"))
        pelr = ctx.enter_context(tc.tile_pool(name="pelr", bufs=2, space="PSUM"))

        # bf16 matmuls tolerate mixed producers: DMA loads feed PE directly
        w_sb = [wpool.tile([128, FEAT], BF16, tag=f"w{i}", name=f"w{i}")
                for i in range(KT)]
        for i in range(KT):
            nc.sync.dma_start(w_sb[i][:], wfc[i * 128:(i + 1) * 128, :])
        attn_sb = wpool.tile([128, KT, 2 * H], BF16, tag="at", name="at")
        nc.gpsimd.dma_start(attn_sb[:],
                            attnb.rearrange("(f p) h -> p f h", p=128))
        biasT_sb = wpool.tile([128, 4], F32, tag="bt", name="bt")
        nc.gpsimd.dma_start(biasT_sb[:], biasT[:])
        elrc_sb = wpool.tile([2 * H, 1], F32, tag="ec", name="ec")
        nc.gpsimd.dma_start(elrc_sb[:], elrc[:])

        for nchk in range(NCH):
            c0 = nchk * 512
            tT_sb = [cpool.tile([128, 512], BF16, tag=f"tt{i}", name=f"tt{i}")
                     for i in range(KT)]
            for i in range(KT):
                nc.sync.dma_start(
                    tT_sb[i][:], textT[i * 128:(i + 1) * 128, c0:c0 + 512])

            # hT[f, n] = sum_d wfc[d, f] * textT[d, n] ; emit bf16 per ft tile
            hb = [cpool.tile([128, 512], BF16, tag=f"hb{i}", name=f"hb{i}")
                  for i in range(KT)]
            for ft in range(KT):
                p = pmm.tile([128, 512], F32, tag="pmm", name="pmm")
                for dt in range(KT):
                    nc.tensor.matmul(
                        p[:],
                        w_sb[dt][:, ft * 128:(ft + 1) * 128],
                        tT_sb[dt][:],
                        start=(dt == 0), stop=(dt == KT - 1))
                nc.scalar.activation(hb[ft][:], p[:], ACT.Identity,
                                     bias=biasT_sb[:, ft:ft + 1])
                nc.gpsimd.dma_start(
                    tableT[ft * 128:(ft + 1) * 128, c0:c0 + 512], hb[ft][:])

            # elrT[c, n] = sum_f attn[f, c] * hT[f, n]
            pe = pelr.tile([2 * H, 512], F32, tag="pelr", name="pelr")
            for ft in range(KT):
                nc.tensor.matmul(
                    pe[:], attn_sb[:, ft, :], hb[ft][:],
                    start=(ft == 0), stop=(ft == KT - 1))
            elr_sb = hpool.tile([2 * H, 512], F32, tag="elr", name="elr")
            nc.vector.tensor_scalar(elr_sb[:], pe[:], elrc_sb[:], None,
                                    op0=ALU.subtract)
            nc.gpsimd.dma_start(elrT[:, c0:c0 + 512], elr_sb[:])
    nc.compile()
    return nc


# ----------------------------------------------------------------------------
# Launch B: edge-softmax aggregation, dst-sharded.
# ----------------------------------------------------------------------------

def build_phase_b(s_max: int):
    SM = s_max

    nc = bacc.Bacc("TRN2", target_bir_lowering=False, debug=False,
                   enable_asserts=False, num_devices=NC)
    ebuf = nc.dram_tensor("ebuf", [BPC * 128, SM * FEAT], BF16,
                          kind="ExternalInput").ap()
    FP8 = mybir.dt.float8e4
    msk_in = nc.dram_tensor("msk", [BPC * 128, SM * 128], FP8,
                            kind="ExternalInput").ap()
    el_in = nc.dram_tensor("elin", [128, BPC * SM * H], BF16,
                           kind="ExternalInput").ap()
    er_in = nc.dram_tensor("erin", [128, BPC * SM * H], BF16,
                           kind="ExternalInput").ap()
    out = nc.dram_tensor("out", [NPC, FEAT], BF16, kind="ExternalOutput").ap()
    I32 = mybir.dt.int32

    with tile.TileContext(nc) as tc, ExitStack() as ctx:
        cpool = ctx.enter_context(tc.tile_pool(name="c", bufs=1))
        gpool = ctx.enter_context(tc.tile_pool(name="g", bufs=5))
        mpool = ctx.enter_context(tc.tile_pool(name="m", bufs=3))
        rpool = ctx.enter_context(tc.tile_pool(name="r", bufs=2))
        wpool = ctx.enter_context(tc.tile_pool(name="wk", bufs=3))
        opool = ctx.enter_context(tc.tile_pool(name="o", bufs=2))
        pfeat = ctx.enter_context(tc.tile_pool(name="pf", bufs=3, space="PSUM"))
        pden = ctx.enter_context(tc.tile_pool(name="pd", bufs=3, space="PSUM"))

        el_sb = cpool.tile([128, BPC, SM, H], BF16, tag="el", name="el")
        nc.sync.dma_start(el_sb[:], el_in.rearrange("p (b s h) -> p b s h",
                                                    b=BPC, s=SM))
        er_sb = cpool.tile([128, BPC, SM, H], BF16, tag="er", name="er")
        nc.sync.dma_start(er_sb[:], er_in.rearrange("p (b s h) -> p b s h",
                                                    b=BPC, s=SM))

        # per-edge weights w = exp(leaky_relu(el[src] + er[dst])), all blocks
        # at once, written twice (packed pairs) so wx can broadcast as int32
        e_all = cpool.tile([128, BPC, SM, H], BF16, tag="e", name="e")
        nc.vector.tensor_tensor(e_all[:], el_sb[:], er_sb[:], op=ALU.add)
        lk_all = cpool.tile([128, BPC, SM, H], BF16, tag="lk", name="lk")
        nc.vector.tensor_scalar_mul(lk_all[:], e_all[:], NEG)
        nc.vector.tensor_max(lk_all[:], lk_all[:], e_all[:])
        wg2 = cpool.tile([128, BPC, SM, H, 2], BF16, tag="wg", name="wg")
        for rep in range(2):
            nc.scalar.activation(wg2[:, :, :, :, rep], lk_all[:], ACT.Exp)

        def block_front(b):
            g_sb = gpool.tile([128, SM, FEAT], BF16, tag="g", name="g")
            nc.gpsimd.dma_start(
                g_sb[:], ebuf[b * 128:(b + 1) * 128, :].rearrange(
                    "p (s f) -> p s f", s=SM))
            # one-hot dst masks, precomputed on host
            m_sb = mpool.tile([128, SM, 128], FP8, tag="m", name="m")
            nc.sync.dma_start(
                m_sb[:], msk_in[b * 128:(b + 1) * 128, :].rearrange(
                    "p (s j) -> p s j", s=SM))

            # materialize w densely on the scalar engine (packed-int32
            # broadcast copy), freeing the DVE for the big multiply
            wx = rpool.tile([128, SM, H, DH], BF16, tag="wx", name="wx")
            wgi = wg2[:, b].bitcast(I32)
            nc.vector.tensor_copy(
                wx[:].bitcast(I32),
                wgi.to_broadcast((128, SM, H, DH // 2)))
            rh = rpool.tile([128, SM, FEAT], BF16, tag="rh", name="rh")
            nc.vector.tensor_tensor(
                rh[:], g_sb[:], wx[:].rearrange("a s h d -> a s (h d)"),
                op=ALU.mult)

            # masked-matmul aggregation + denominators
            pf = pfeat.tile([128, FEAT], F32, tag="pf", name="pf")
            pd = pden.tile([128, H], F32, tag="pd", name="pd")
            for sbt in range(SM):
                st, sp = (sbt == 0), (sbt == SM - 1)
                nc.tensor.matmul(pf[:], m_sb[:, sbt, :], rh[:, sbt],
                                 start=st, stop=sp)
                nc.tensor.matmul(pd[:], m_sb[:, sbt, :], wg2[:, b, sbt, :, 0],
                                 start=st, stop=sp)
            return pf, pd

        def block_epilogue(b, pf, pd):
            den_sb = wpool.tile([128, H], F32, tag="den", name="den")
            nc.scalar.activation(den_sb[:], pd[:], ACT.Copy)
            rec_sb = wpool.tile([128, H], F32, tag="rec", name="rec")
            nc.vector.reciprocal(rec_sb[:], den_sb[:])
            o_sb = opool.tile([128, FEAT], BF16, tag="o", name="o")
            for h in range(H):
                nc.scalar.activation(
                    o_sb[:, h * DH:(h + 1) * DH], pf[:, h * DH:(h + 1) * DH],
                    ACT.Copy, scale=rec_sb[:, h:h + 1])
            nc.gpsimd.dma_start(out[b * 128:(b + 1) * 128, :], o_sb[:])

        # software pipeline: block b's epilogue is emitted after block b+1's
        # front so no engine stream stalls on the PSUM accumulation
        prev = None
        for b in range(BPC):
            cur = block_front(b)
            if prev is not None:
                block_epilogue(b - 1, *prev)
            prev = cur
        block_epilogue(BPC - 1, *prev)
    nc.compile()
    return nc


# ----------------------------------------------------------------------------
# Host side
# ----------------------------------------------------------------------------

def _preprocess(src, dst):
    """Relabel nodes so 128-dst blocks are edge-balanced (snake by degree,
    then swap-refine toward perfectly equal block sums); build per-edge
    block layouts (edge position = subtile*128 + partition)."""
    import collections

    deg = np.bincount(dst, minlength=N)
    order = np.argsort(-deg, kind="stable")
    ranks = np.arange(N)
    rounds, pos = ranks // NBLK, ranks % NBLK
    blk = np.where(rounds % 2 == 0, pos, NBLK - 1 - pos)
    blk_of_node = np.empty(N, np.int64)
    blk_of_node[order] = blk
    target = len(dst) // NBLK

    bnodes = [collections.defaultdict(set) for _ in range(NBLK)]
    bs = np.zeros(NBLK, np.int64)
    for n in range(N):
        b = blk_of_node[n]
        bnodes[b][int(deg[n])].add(n)
        bs[b] += deg[n]

    def find_swap(hi, lo, delta):
        for da in sorted(bnodes[hi].keys(), reverse=True):
            if bnodes[hi][da] and bnodes[lo].get(da - delta):
                return next(iter(bnodes[hi][da])), next(iter(bnodes[lo][da - delta]))
        return None

    for _ in range(5000):
        hi = int(np.argmax(bs))
        if bs[hi] <= target:
            break
        done = False
        for lo in np.argsort(bs):
            lo = int(lo)
            if bs[lo] >= target:
                break
            dmax = int(min(bs[hi] - target, target - bs[lo]))
            for delta in range(dmax, 0, -1):
                pair = find_swap(hi, lo, delta)
                if pair:
                    a, b_ = pair
                    bnodes[hi][deg[a]].discard(a)
                    bnodes[lo][deg[b_]].discard(b_)
                    bnodes[hi][deg[b_]].add(b_)
                    bnodes[lo][deg[a]].add(a)
                    blk_of_node[a], blk_of_node[b_] = lo, hi
                    bs[hi] -= delta
                    bs[lo] += delta
                    done = True
                    break
            if done:
                break
        if not done:
            break

    eo_n = np.argsort(blk_of_node, kind="stable")
    new_id = np.empty(N, np.int64)
    new_id[eo_n] = np.arange(N)
    bsum = np.bincount(new_id[dst] // 128, minlength=NBLK)
    s_max = int(np.ceil(bsum.max() / 128))
    p_b = s_max * 128
    s2, d2 = new_id[src], new_id[dst]
    eo = np.argsort(d2, kind="stable")
    s2, d2 = s2[eo], d2[eo]
    starts = np.concatenate([[0], np.cumsum(bsum)])
    eblk = d2 // 128
    flatpos = eblk * p_b + (np.arange(len(d2)) - starts[eblk])
    bsrc = np.zeros(NBLK * p_b, np.int64)
    bsrc[flatpos] = s2
    bdst = np.zeros(NBLK * p_b, np.int64)
    bdst[flatpos] = d2
    bcol = np.full(NBLK * p_b, 255.0, np.float32)
    bcol[flatpos] = (d2 % 128).astype(np.float32)
    return (new_id, bsrc.reshape(NBLK, p_b), bdst.reshape(NBLK, p_b),
            bcol.reshape(NBLK, p_b), s_max)


_CACHE = {}


class _Runner:
    """Cached SPMD runner: jits the bass_exec body once per Bass module."""

    def __init__(self, nc):
        install_neuronx_cc_hook()
        self.nc = nc
        part_name = (nc.partition_id_tensor.name
                     if nc.partition_id_tensor else None)
        in_names, out_names, out_avals, zero_outs = [], [], [], []
        for alloc in nc.m.functions[0].allocations:
            if not isinstance(alloc, mybir.MemoryLocationSet):
                continue
            name = alloc.memorylocations[0].name
            if alloc.kind == "ExternalInput":
                if name != part_name:
                    in_names.append(name)
            elif alloc.kind == "ExternalOutput":
                out_names.append(name)
                shape = tuple(alloc.tensor_shape)
                dtype = mybir.dt.np(alloc.dtype)
                out_avals.append(jax.core.ShapedArray(shape, dtype))
                zero_outs.append(np.zeros(shape, dtype))
        self.in_names, self.out_names = in_names, out_names
        self.out_avals, self.zero_outs = out_avals, zero_outs
        n_params, n_outs = len(in_names), len(out_avals)
        all_names = tuple(in_names + out_names
                          + ([part_name] if part_name else []))
        avals = tuple(out_avals)

        def _body(*args):
            operands = list(args)
            if part_name is not None:
                operands.append(partition_id_tensor())
            outs = _bass_exec_p.bind(
                *operands,
                out_avals=avals,
                in_names=all_names,
                out_names=tuple(out_names),
                lowering_input_output_aliases=(),
                sim_require_finite=True,
                sim_require_nnan=True,
                nc=nc,
            )
            return tuple(outs)

        devices = jax.devices()[:NC]
        self.mesh = Mesh(np.asarray(devices), ("core",))
        in_specs = (PartitionSpec("core"),) * (n_params + n_outs)
        out_specs = (PartitionSpec("core"),) * n_outs
        self.fn = jax.jit(
            shard_map(_body, mesh=self.mesh, in_specs=in_specs,
                      out_specs=out_specs, check_rep=False),
            keep_unused=True)

    def prep(self, in_maps):
        """Concatenate per-core inputs along axis 0 (host)."""
        n_params = len(self.in_names)
        concat_in = [
            np.concatenate([in_maps[c][self.in_names[i]] for c in range(NC)],
                           axis=0)
            for i in range(n_params)]
        concat_zeros = [
            np.zeros((NC * z.shape[0], *z.shape[1:]), z.dtype)
            for z in self.zero_outs]
        return concat_in + concat_zeros

    def run_prepped(self, args):
        return self.fn(*args)

    def run(self, in_maps):
        out_arrs = self.fn(*self.prep(in_maps))
        return [
            {name: np.asarray(out_arrs[i]).reshape(NC, *self.out_avals[i].shape)[c]
             for i, name in enumerate(self.out_names)}
            for c in range(NC)]


def _get_kernels(s_max):
    if s_max not in _CACHE:
        _CACHE[s_max] = (_Runner(build_phase_a()), _Runner(build_phase_b(s_max)))
    return _CACHE[s_max]


def kernel(text, weight, fc_w, attn_l, attn_r, bias, src, dst):
    text = np.asarray(text, np.float32)
    weight = np.asarray(weight, np.float32)
    fc_w = np.asarray(fc_w, np.float32)
    attn_l = np.asarray(attn_l, np.float32)
    attn_r = np.asarray(attn_r, np.float32)
    bias = np.asarray(bias, np.float32)
    src = np.asarray(src).astype(np.int64)
    dst = np.asarray(dst).astype(np.int64)

    new_id, bsrc, bdst, bcol, s_max = _preprocess(src, dst)
    orig_for_new = np.empty(N, np.int64)
    orig_for_new[new_id] = np.arange(N)

    run_a, run_b = _get_kernels(s_max)

    # --- launch A ---
    wfc = (weight.astype(np.float64) @ fc_w.astype(np.float64)).astype(BF16NP)
    attn_cat = np.zeros((DIN, 2 * H), np.float32)
    for h in range(H):
        attn_cat[h * DH:(h + 1) * DH, h] = attn_l[h]
        attn_cat[h * DH:(h + 1) * DH, H + h] = attn_r[h]
    attn_b = attn_cat.astype(BF16NP)
    biasT_h = np.ascontiguousarray(bias.reshape(4, 128).T, dtype=np.float32)
    elrc_h = (bias @ attn_cat).reshape(2 * H, 1).astype(np.float32)
    text_flat = text.reshape(N, DIN)
    in_maps_a = []
    for c in range(NC):
        rows = orig_for_new[c * NPC:(c + 1) * NPC]
        textT = np.ascontiguousarray(text_flat[rows].T).astype(BF16NP)
        in_maps_a.append({"textT": textT, "wfc": wfc, "attnb": attn_b,
                          "biasT": biasT_h, "elrc": elrc_h})
    res_a = run_a.run(in_maps_a)

    # node-major table / el / er in new-id space
    table_full = np.concatenate(
        [np.ascontiguousarray(r["tableT"].T) for r in res_a], axis=0)
    elr_full = np.concatenate([r["elrT"].T for r in res_a], axis=0)
    el_full = np.ascontiguousarray(elr_full[:, :H]).astype(BF16NP)
    er_full = np.ascontiguousarray(elr_full[:, H:]).astype(BF16NP)

    # --- host expansion: node table -> per-edge buffers ---
    # bsrc[blk, s*128+p] -> layout [blk, p, s]
    idx_ps = bsrc.reshape(NBLK, s_max, 128).transpose(0, 2, 1)
    ebuf_all = table_full[idx_ps].reshape(NBLK, 128, s_max * FEAT)
    el_e = el_full[idx_ps]                               # [NBLK,128,s_max,H]
    er_e = er_full[bdst.reshape(NBLK, s_max, 128).transpose(0, 2, 1)]
    # one-hot dst masks [blk, p, s, j]
    msk_all = (bcol.reshape(NBLK, s_max, 128).transpose(0, 2, 1)[:, :, :, None]
               == np.arange(128, dtype=np.float32)).astype(ml_dtypes.float8_e4m3)
    msk_all = msk_all.reshape(NBLK, 128, s_max * 128)

    in_maps_b = []
    for c in range(NC):
        blks = slice(c * BPC, (c + 1) * BPC)
        elin = np.ascontiguousarray(
            el_e[blks].transpose(1, 0, 2, 3).reshape(128, BPC * s_max * H))
        erin = np.ascontiguousarray(
            er_e[blks].transpose(1, 0, 2, 3).reshape(128, BPC * s_max * H))
        in_maps_b.append({
            "ebuf": ebuf_all[blks].reshape(BPC * 128, s_max * FEAT),
            "msk": msk_all[blks].reshape(BPC * 128, s_max * 128),
            "elin": elin, "erin": erin})
    res_b = run_b.run(in_maps_b)

    out_new = np.concatenate([r["out"].astype(np.float32) for r in res_b],
                             axis=0)
    result = out_new[new_id].reshape(B, L, H * DH).astype(np.float32)

    global _LAST_ARGS
    _LAST_ARGS = (run_a, in_maps_a, run_b, in_maps_b)
    return result


_LAST_ARGS = None


# revision 20
# speedup vs baseline: 2.2750x; 1.1443x over previous
"""Trainium2 Bass kernel for nn_DglGraphAttentionNetwork (GAT layer over a
random graph, B=16, L=1024, DIN=512, H=4 heads, DH=128).

Strategy (8 NeuronCores, SPMD, two launches + host glue):
  Launch A (data-parallel over nodes): each core projects its 2048 nodes
    h = text @ (W @ fc_w)  (weight product prefolded on host, f32r matmuls)
    and el/er = h . attn_{l,r}. Outputs stay feature-major (tableT [512,2048]
    bf16, elrT [8,2048] f32) so the device does no transposes.
  Host: transposes/concats the 8 table slices, then expands the node table
    into per-edge order (the "gather" is a host permutation): each core
    receives an edge buffer ebuf[block, 128, s_max*512] plus per-edge
    el[src], er[dst] slices. A device dma_gather is descriptor-rate-bound
    (~8ns/row on GpSimd), while plain DMA streams at the full 360GB/s.
  Launch B (dst-sharded): 128-dst blocks, 16 per core. Per block: DMA the
    edge rows, build one-hot dst masks with 4x-mode tensor_scalar(is_equal),
    compute per-edge softmax weights w = exp(leaky(el+er)) on ACT, weight
    the messages on DVE (rh = w*h), and accumulate per-destination sums and
    denominators as masked matmuls in PSUM.
"""

import os
import sys

sys.path.insert(0, "/opt/trn_rl_repo")

from contextlib import ExitStack

import numpy as np
import ml_dtypes

import jax
from jax.sharding import Mesh, PartitionSpec
from jax.experimental.shard_map import shard_map

try:
    jax.config.update("jax_compilation_cache_dir", "/tmp/gat_jax_cache")
    jax.config.update("jax_persistent_cache_min_compile_time_secs", 1.0)
    jax.config.update("jax_persistent_cache_min_entry_size_bytes", -1)
except Exception:
    pass

import concourse.bass as bass
import concourse.bacc as bacc
import concourse.mybir as mybir
import concourse.tile as tile
from concourse.bass2jax import _bass_exec_p, install_neuronx_cc_hook, partition_id_tensor

F32 = mybir.dt.float32
F32R = mybir.dt.float32r
BF16 = mybir.dt.bfloat16
BF16NP = ml_dtypes.bfloat16

B, L, DIN = 16, 1024, 512
H, DH = 4, 128
N = B * L           # 16384 nodes
NC = 8              # cores
NPC = N // NC       # 2048 nodes per core
NBLK = 128          # destination blocks of 128 nodes
BPC = NBLK // NC    # 16 blocks per core
NEG = 0.2           # leaky_relu slope
FEAT = H * DH       # 512

ACT = mybir.ActivationFunctionType
ALU = mybir.AluOpType


# ----------------------------------------------------------------------------
# Launch A: projection. Per core: textT [512, 2048] -> tableT [512, 2048] bf16,
# elrT [8, 2048] f32.
# ----------------------------------------------------------------------------

def build_phase_a():
    nc = bacc.Bacc("TRN2", target_bir_lowering=False, debug=False,
                   enable_asserts=False, num_devices=NC)
    textT = nc.dram_tensor("textT", [DIN, NPC], BF16, kind="ExternalInput").ap()
    wfc = nc.dram_tensor("wfc", [DIN, FEAT], BF16, kind="ExternalInput").ap()
    attnb = nc.dram_tensor("attnb", [DIN, 2 * H], BF16, kind="ExternalInput").ap()
    biasT = nc.dram_tensor("biasT", [128, 4], F32, kind="ExternalInput").ap()
    elrc = nc.dram_tensor("elrc", [2 * H, 1], F32, kind="ExternalInput").ap()
    tableT = nc.dram_tensor("tableT", [FEAT, NPC], BF16, kind="ExternalOutput").ap()
    elrT = nc.dram_tensor("elrT", [2 * H, NPC], F32, kind="ExternalOutput").ap()

    KT = DIN // 128    # 4 contraction tiles
    NCH = NPC // 512   # 4 node chunks of 512

    with tile.TileContext(nc) as tc, ExitStack() as ctx:
        wpool = ctx.enter_context(tc.tile_pool(name="w", bufs=1))
        cpool = ctx.enter_context(tc.tile_pool(name="c", bufs=2))
        hpool = ctx.enter_context(tc.tile_pool(name="h", bufs=2))
        pmm = ctx.enter_context(tc.tile_pool(name="pmm", bufs=4, space="PSUM"))
        pelr = ctx.enter_context(tc.tile_pool(name="pelr", bufs=2, space="PSUM"))

        # bf16 matmuls tolerate mixed producers: DMA loads feed PE directly
        w_sb = [wpool.tile([128, FEAT], BF16, tag=f"w{i}", name=f"w{i}")
                for i in range(KT)]
        for i in range(KT):
            nc.sync.dma_start(w_sb[i][:], wfc[i * 128:(i + 1) * 128, :])
        attn_sb = wpool.tile([128, KT, 2 * H], BF16, tag="at", name="at")
        nc.gpsimd.dma_start(attn_sb[:],
                            attnb.rearrange("(f p) h -> p f h", p=128))
        biasT_sb = wpool.tile([128, 4], F32, tag="bt", name="bt")
        nc.gpsimd.dma_start(biasT_sb[:], biasT[:])
        elrc_sb = wpool.tile([2 * H, 1], F32, tag="ec", name="ec")
        nc.gpsimd.dma_start(elrc_sb[:], elrc[:])

        for nchk in range(NCH):
            c0 = nchk * 512
            tT_sb = [cpool.tile([128, 512], BF16, tag=f"tt{i}", name=f"tt{i}")
                     for i in range(KT)]
            for i in range(KT):
                nc.sync.dma_start(
                    tT_sb[i][:], textT[i * 128:(i + 1) * 128, c0:c0 + 512])

            # hT[f, n] = sum_d wfc[d, f] * textT[d, n] ; emit bf16 per ft tile
            hb = [cpool.tile([128, 512], BF16, tag=f"hb{i}", name=f"hb{i}")
                  for i in range(KT)]
            for ft in range(KT):
                p = pmm.tile([128, 512], F32, tag="pmm", name="pmm")
                for dt in range(KT):
                    nc.tensor.matmul(
                        p[:],
                        w_sb[dt][:, ft * 128:(ft + 1) * 128],
                        tT_sb[dt][:],
                        start=(dt == 0), stop=(dt == KT - 1))
                nc.scalar.activation(hb[ft][:], p[:], ACT.Identity,
                                     bias=biasT_sb[:, ft:ft + 1])
                nc.gpsimd.dma_start(
                    tableT[ft * 128:(ft + 1) * 128, c0:c0 + 512], hb[ft][:])

            # elrT[c, n] = sum_f attn[f, c] * hT[f, n]
            pe = pelr.tile([2 * H, 512], F32, tag="pelr", name="pelr")
            for ft in range(KT):
                nc.tensor.matmul(
                    pe[:], attn_sb[:, ft, :], hb[ft][:],
                    start=(ft == 0), stop=(ft == KT - 1))
            elr_sb = hpool.tile([2 * H, 512], F32, tag="elr", name="elr")
            nc.vector.tensor_scalar(elr_sb[:], pe[:], elrc_sb[:], None,
                                    op0=ALU.subtract)
            nc.gpsimd.dma_start(elrT[:, c0:c0 + 512], elr_sb[:])
    nc.compile()
    return nc


# ----------------------------------------------------------------------------
# Launch B: edge-softmax aggregation, dst-sharded.
# ----------------------------------------------------------------------------

def build_phase_b(s_max: int):
    SM = s_max

    nc = bacc.Bacc("TRN2", target_bir_lowering=False, debug=False,
                   enable_asserts=False, num_devices=NC)
    ebuf = nc.dram_tensor("ebuf", [BPC * 128, SM * FEAT], BF16,
                          kind="ExternalInput").ap()
    FP8 = mybir.dt.float8e4
    msk_in = nc.dram_tensor("msk", [BPC * 128, SM * 128], FP8,
                            kind="ExternalInput").ap()
    el_in = nc.dram_tensor("elin", [128, BPC * SM * H], BF16,
                           kind="ExternalInput").ap()
    er_in = nc.dram_tensor("erin", [128, BPC * SM * H], BF16,
                           kind="ExternalInput").ap()
    out = nc.dram_tensor("out", [NPC, FEAT], BF16, kind="ExternalOutput").ap()
    I32 = mybir.dt.int32

    with tile.TileContext(nc) as tc, ExitStack() as ctx:
        cpool = ctx.enter_context(tc.tile_pool(name="c", bufs=1))
        gpool = ctx.enter_context(tc.tile_pool(name="g", bufs=5))
        mpool = ctx.enter_context(tc.tile_pool(name="m", bufs=3))
        rpool = ctx.enter_context(tc.tile_pool(name="r", bufs=2))
        wpool = ctx.enter_context(tc.tile_pool(name="wk", bufs=3))
        opool = ctx.enter_context(tc.tile_pool(name="o", bufs=2))
        pfeat = ctx.enter_context(tc.tile_pool(name="pf", bufs=3, space="PSUM"))
        pden = ctx.enter_context(tc.tile_pool(name="pd", bufs=3, space="PSUM"))

        el_sb = cpool.tile([128, BPC, SM, H], BF16, tag="el", name="el")
        nc.sync.dma_start(el_sb[:], el_in.rearrange("p (b s h) -> p b s h",
                                                    b=BPC, s=SM))
        er_sb = cpool.tile([128, BPC, SM, H], BF16, tag="er", name="er")
        nc.sync.dma_start(er_sb[:], er_in.rearrange("p (b s h) -> p b s h",
                                                    b=BPC, s=SM))

        # per-edge weights w = exp(leaky_relu(el[src] + er[dst])), all blocks
        # at once, written twice (packed pairs) so wx can broadcast as int32
        e_all = cpool.tile([128, BPC, SM, H], BF16, tag="e", name="e")
        nc.vector.tensor_tensor(e_all[:], el_sb[:], er_sb[:], op=ALU.add)
        lk_all = cpool.tile([128, BPC, SM, H], BF16, tag="lk", name="lk")
        nc.vector.tensor_scalar_mul(lk_all[:], e_all[:], NEG)
        nc.vector.tensor_max(lk_all[:], lk_all[:], e_all[:])
        wg2 = cpool.tile([128, BPC, SM, H, 2], BF16, tag="wg", name="wg")
        for rep in range(2):
            nc.scalar.activation(wg2[:, :, :, :, rep], lk_all[:], ACT.Exp)

        def block_front(b):
            g_sb = gpool.tile([128, SM, FEAT], BF16, tag="g", name="g")
            nc.gpsimd.dma_start(
                g_sb[:], ebuf[b * 128:(b + 1) * 128, :].rearrange(
                    "p (s f) -> p s f", s=SM))
            # one-hot dst masks, precomputed on host
            m_sb = mpool.tile([128, SM, 128], FP8, tag="m", name="m")
            nc.sync.dma_start(
                m_sb[:], msk_in[b * 128:(b + 1) * 128, :].rearrange(
                    "p (s j) -> p s j", s=SM))

            # materialize w densely on the scalar engine (packed-int32
            # broadcast copy), freeing the DVE for the big multiply
            wx = rpool.tile([128, SM, H, DH], BF16, tag="wx", name="wx")
            wgi = wg2[:, b].bitcast(I32)
            nc.vector.tensor_copy(
                wx[:].bitcast(I32),
                wgi.to_broadcast((128, SM, H, DH // 2)))
            rh = rpool.tile([128, SM, FEAT], BF16, tag="rh", name="rh")
            nc.vector.tensor_tensor(
                rh[:], g_sb[:], wx[:].rearrange("a s h d -> a s (h d)"),
                op=ALU.mult)

            # masked-matmul aggregation + denominators
            pf = pfeat.tile([128, FEAT], F32, tag="pf", name="pf")
            pd = pden.tile([128, H], F32, tag="pd", name="pd")
            for sbt in range(SM):
                st, sp = (sbt == 0), (sbt == SM - 1)
                nc.tensor.matmul(pf[:], m_sb[:, sbt, :], rh[:, sbt],
                                 start=st, stop=sp)
                nc.tensor.matmul(pd[:], m_sb[:, sbt, :], wg2[:, b, sbt, :, 0],
                                 start=st, stop=sp)
            return pf, pd

        def block_epilogue(b, pf, pd):
            den_sb = wpool.tile([128, H], F32, tag="den", name="den")
            nc.scalar.activation(den_sb[:], pd[:], ACT.Copy)
            rec_sb = wpool.tile([128, H], F32, tag="rec", name="rec")
            nc.vector.reciprocal(rec_sb[:], den_sb[:])
            o_sb = opool.tile([128, FEAT], BF16, tag="o", name="o")
            for h in range(H):
                nc.scalar.activation(
                    o_sb[:, h * DH:(h + 1) * DH], pf[:, h * DH:(h + 1) * DH],
                    ACT.Copy, scale=rec_sb[:, h:h + 1])
            nc.scalar.dma_start(out[b * 128:(b + 1) * 128, :], o_sb[:])

        # software pipeline: block b's epilogue is emitted after block b+1's
        # front so no engine stream stalls on the PSUM accumulation
        prev = None
        for b in range(BPC):
            cur = block_front(b)
            if prev is not None:
                block_epilogue(b - 1, *prev)
            prev = cur
        block_epilogue(BPC - 1, *prev)
    nc.compile()
    return nc


# ----------------------------------------------------------------------------
# Host side
# ----------------------------------------------------------------------------

def _preprocess(src, dst):
    """Relabel nodes so 128-dst blocks are edge-balanced (snake by degree,
    then swap-refine toward perfectly equal block sums); build per-edge
    block layouts (edge position = subtile*128 + partition)."""
    import collections

    deg = np.bincount(dst, minlength=N)
    order = np.argsort(-deg, kind="stable")
    ranks = np.arange(N)
    rounds, pos = ranks // NBLK, ranks % NBLK
    blk = np.where(rounds % 2 == 0, pos, NBLK - 1 - pos)
    blk_of_node = np.empty(N, np.int64)
    blk_of_node[order] = blk
    target = len(dst) // NBLK

    bnodes = [collections.defaultdict(set) for _ in range(NBLK)]
    bs = np.zeros(NBLK, np.int64)
    for n in range(N):
        b = blk_of_node[n]
        bnodes[b][int(deg[n])].add(n)
        bs[b] += deg[n]

    def find_swap(hi, lo, delta):
        for da in sorted(bnodes[hi].keys(), reverse=True):
            if bnodes[hi][da] and bnodes[lo].get(da - delta):
                return next(iter(bnodes[hi][da])), next(iter(bnodes[lo][da - delta]))
        return None

    for _ in range(5000):
        hi = int(np.argmax(bs))
        if bs[hi] <= target:
            break
        done = False
        for lo in np.argsort(bs):
            lo = int(lo)
            if bs[lo] >= target:
                break
            dmax = int(min(bs[hi] - target, target - bs[lo]))
            for delta in range(dmax, 0, -1):
                pair = find_swap(hi, lo, delta)
                if pair:
                    a, b_ = pair
                    bnodes[hi][deg[a]].discard(a)
                    bnodes[lo][deg[b_]].discard(b_)
                    bnodes[hi][deg[b_]].add(b_)
                    bnodes[lo][deg[a]].add(a)
                    blk_of_node[a], blk_of_node[b_] = lo, hi
                    bs[hi] -= delta
                    bs[lo] += delta
                    done = True
                    break
            if done:
                break
        if not done:
            break

    eo_n = np.argsort(blk_of_node, kind="stable")
    new_id = np.empty(N, np.int64)
    new_id[eo_n] = np.arange(N)
    bsum = np.bincount(new_id[dst] // 128, minlength=NBLK)
    s_max = int(np.ceil(bsum.max() / 128))
    p_b = s_max * 128
    s2, d2 = new_id[src], new_id[dst]
    eo = np.argsort(d2, kind="stable")
    s2, d2 = s2[eo], d2[eo]
    starts = np.concatenate([[0], np.cumsum(bsum)])
    eblk = d2 // 128
    flatpos = eblk * p_b + (np.arange(len(d2)) - starts[eblk])
    bsrc = np.zeros(NBLK * p_b, np.int64)
    bsrc[flatpos] = s2
    bdst = np.zeros(NBLK * p_b, np.int64)
    bdst[flatpos] = d2
    bcol = np.full(NBLK * p_b, 255.0, np.float32)
    bcol[flatpos] = (d2 % 128).astype(np.float32)
    return (new_id, bsrc.reshape(NBLK, p_b), bdst.reshape(NBLK, p_b),
            bcol.reshape(NBLK, p_b), s_max)


_CACHE = {}


class _Runner:
    """Cached SPMD runner: jits the bass_exec body once per Bass module."""

    def __init__(self, nc):
        install_neuronx_cc_hook()
        self.nc = nc
        part_name = (nc.partition_id_tensor.name
                     if nc.partition_id_tensor else None)
        in_names, out_names, out_avals, zero_outs = [], [], [], []
        for alloc in nc.m.functions[0].allocations:
            if not isinstance(alloc, mybir.MemoryLocationSet):
                continue
            name = alloc.memorylocations[0].name
            if alloc.kind == "ExternalInput":
                if name != part_name:
                    in_names.append(name)
            elif alloc.kind == "ExternalOutput":
                out_names.append(name)
                shape = tuple(alloc.tensor_shape)
                dtype = mybir.dt.np(alloc.dtype)
                out_avals.append(jax.core.ShapedArray(shape, dtype))
                zero_outs.append(np.zeros(shape, dtype))
        self.in_names, self.out_names = in_names, out_names
        self.out_avals, self.zero_outs = out_avals, zero_outs
        n_params, n_outs = len(in_names), len(out_avals)
        all_names = tuple(in_names + out_names
                          + ([part_name] if part_name else []))
        avals = tuple(out_avals)

        def _body(*args):
            operands = list(args)
            if part_name is not None:
                operands.append(partition_id_tensor())
            outs = _bass_exec_p.bind(
                *operands,
                out_avals=avals,
                in_names=all_names,
                out_names=tuple(out_names),
                lowering_input_output_aliases=(),
                sim_require_finite=True,
                sim_require_nnan=True,
                nc=nc,
            )
            return tuple(outs)

        devices = jax.devices()[:NC]
        self.mesh = Mesh(np.asarray(devices), ("core",))
        in_specs = (PartitionSpec("core"),) * (n_params + n_outs)
        out_specs = (PartitionSpec("core"),) * n_outs
        self.fn = jax.jit(
            shard_map(_body, mesh=self.mesh, in_specs=in_specs,
                      out_specs=out_specs, check_rep=False),
            keep_unused=True)

    def prep(self, in_maps):
        """Concatenate per-core inputs along axis 0 (host)."""
        n_params = len(self.in_names)
        concat_in = [
            np.concatenate([in_maps[c][self.in_names[i]] for c in range(NC)],
                           axis=0)
            for i in range(n_params)]
        concat_zeros = [
            np.zeros((NC * z.shape[0], *z.shape[1:]), z.dtype)
            for z in self.zero_outs]
        return concat_in + concat_zeros

    def run_prepped(self, args):
        return self.fn(*args)

    def run(self, in_maps):
        out_arrs = self.fn(*self.prep(in_maps))
        return [
            {name: np.asarray(out_arrs[i]).reshape(NC, *self.out_avals[i].shape)[c]
             for i, name in enumerate(self.out_names)}
            for c in range(NC)]


def _get_kernels(s_max):
    if s_max not in _CACHE:
        _CACHE[s_max] = (_Runner(build_phase_a()), _Runner(build_phase_b(s_max)))
    return _CACHE[s_max]


def kernel(text, weight, fc_w, attn_l, attn_r, bias, src, dst):
    text = np.asarray(text, np.float32)
    weight = np.asarray(weight, np.float32)
    fc_w = np.asarray(fc_w, np.float32)
    attn_l = np.asarray(attn_l, np.float32)
    attn_r = np.asarray(attn_r, np.float32)
    bias = np.asarray(bias, np.float32)
    src = np.asarray(src).astype(np.int64)
    dst = np.asarray(dst).astype(np.int64)

    new_id, bsrc, bdst, bcol, s_max = _preprocess(src, dst)
    orig_for_new = np.empty(N, np.int64)
    orig_for_new[new_id] = np.arange(N)

    run_a, run_b = _get_kernels(s_max)

    # --- launch A ---
    wfc = (weight.astype(np.float64) @ fc_w.astype(np.float64)).astype(BF16NP)
    attn_cat = np.zeros((DIN, 2 * H), np.float32)
    for h in range(H):
        attn_cat[h * DH:(h + 1) * DH, h] = attn_l[h]
        attn_cat[h * DH:(h + 1) * DH, H + h] = attn_r[h]
    attn_b = attn_cat.astype(BF16NP)
    biasT_h = np.ascontiguousarray(bias.reshape(4, 128).T, dtype=np.float32)
    elrc_h = (bias @ attn_cat).reshape(2 * H, 1).astype(np.float32)
    text_flat = text.reshape(N, DIN)
    in_maps_a = []
    for c in range(NC):
        rows = orig_for_new[c * NPC:(c + 1) * NPC]
        textT = np.ascontiguousarray(text_flat[rows].T).astype(BF16NP)
        in_maps_a.append({"textT": textT, "wfc": wfc, "attnb": attn_b,
                          "biasT": biasT_h, "elrc": elrc_h})
    res_a = run_a.run(in_maps_a)

    # node-major table / el / er in new-id space
    table_full = np.concatenate(
        [np.ascontiguousarray(r["tableT"].T) for r in res_a], axis=0)
    elr_full = np.concatenate([r["elrT"].T for r in res_a], axis=0)
    el_full = np.ascontiguousarray(elr_full[:, :H]).astype(BF16NP)
    er_full = np.ascontiguousarray(elr_full[:, H:]).astype(BF16NP)

    # --- host expansion: node table -> per-edge buffers ---
    # bsrc[blk, s*128+p] -> layout [blk, p, s]
    idx_ps = bsrc.reshape(NBLK, s_max, 128).transpose(0, 2, 1)
    ebuf_all = table_full[idx_ps].reshape(NBLK, 128, s_max * FEAT)
    el_e = el_full[idx_ps]                               # [NBLK,128,s_max,H]
    er_e = er_full[bdst.reshape(NBLK, s_max, 128).transpose(0, 2, 1)]
    # one-hot dst masks [blk, p, s, j]
    msk_all = (bcol.reshape(NBLK, s_max, 128).transpose(0, 2, 1)[:, :, :, None]
               == np.arange(128, dtype=np.float32)).astype(ml_dtypes.float8_e4m3)
    msk_all = msk_all.reshape(NBLK, 128, s_max * 128)

    in_maps_b = []
    for c in range(NC):
        blks = slice(c * BPC, (c + 1) * BPC)
        elin = np.ascontiguousarray(
            el_e[blks].transpose(1, 0, 2, 3).reshape(128, BPC * s_max * H))
        erin = np.ascontiguousarray(
            er_e[blks].transpose(1, 0, 2, 3).reshape(128, BPC * s_max * H))
        in_maps_b.append({
            "ebuf": ebuf_all[blks].reshape(BPC * 128, s_max * FEAT),
            "msk": msk_all[blks].reshape(BPC * 128, s_max * 128),
            "elin": elin, "erin": erin})
    res_b = run_b.run(in_maps_b)

    out_new = np.concatenate([r["out"].astype(np.float32) for r in res_b],
                             axis=0)
    result = out_new[new_id].reshape(B, L, H * DH).astype(np.float32)

    global _LAST_ARGS
    _LAST_ARGS = (run_a, in_maps_a, run_b, in_maps_b)
    return result


_LAST_ARGS = None
